# revision 1
# baseline (speedup 1.0000x reference)
"""BatchGGNNEncoder Trainium2 kernel: 8-core SPMD, dst-sharded message passing.

Full inputs in, full output out. Internally:
  - core c owns nodes [c*4096, (c+1)*4096) = graphs [4c, 4c+4) (data parallel).
  - aggregate-first GGNN layer:
        A_t[v] = sum_{e: dst=v, type=t} h[src_e]         (one-hot matmuls, PSUM)
        m      = sum_t A_t @ Wm[t].T + counts_t * bm[t]  (dense matmuls)
        h      = GRU(m, h)                               (matmuls + DVE/ACT)
  - h table (bf16, node-major) lives in DRAM, AllGathered across cores per layer;
    per-edge h[src] rows fetched with dma_gather (the kernel's critical path:
    ~8.4ns/edge of Q7 descriptor generation).
  - staging (transpose to node-major + DMA) for layer l+1's table is fused into
    layer l's per-graph GRU tail so the AllGather fires as early as possible.
  - nodes are permuted within each graph to balance (type, 128-dst-window) group
    sizes so the compiled program structure is identical on all 8 cores.
"""
import numpy as np
import ml_dtypes

import concourse.bass as bass
import concourse.bacc as bacc
import concourse.mybir as mybir
import concourse.tile as tile
from concourse.bass_utils import run_bass_kernel_spmd

BF16 = ml_dtypes.bfloat16

# problem constants (hardcoded per harness contract)
MAXN, F, H, T, L = 1024, 215, 256, 8, 3
NCORES = 8
WIN = 128                     # dst window (one-hot free width)
WPG = MAXN // WIN             # 8 windows per graph
GSZ = 8                       # chunks per dma_gather (8*128=1024 idxs; the SWDGE
                              # ring holds 64 m2s + 64 s2m pairs per engine, so
                              # 1024 idxs is the hard maximum per call)


def _balance_graph(deg):
    """Assign 1024 nodes (deg: [1024, T] type-degrees) to 8 windows of 128.
    Window WPG-1 takes the heaviest 128 nodes (the graph's excess, ~3 chunks
    per type); the remaining 896 are balanced across windows 0..WPG-2 under a
    hard 256 cap per type (2 chunks), with real slack since the heavy nodes
    are gone. Keeps cross-core max budgets at 2 for most groups."""
    tot = deg.sum(1)
    order = np.argsort(-tot, kind="stable")
    last = WPG - 1
    wsum = np.zeros((WPG, T), np.float64)
    wcnt = np.zeros(WPG, np.int64)
    members = [[] for _ in range(WPG)]
    CAP, CAP7 = 256.0, 381.0
    rest = []
    for nd in order:
        if wcnt[last] < 128 and ((wsum[last] + deg[nd]) <= CAP7).all():
            members[last].append(nd)
            wsum[last] += deg[nd]
            wcnt[last] += 1
        else:
            rest.append(nd)
    for nd in rest:
        d = deg[nd]
        ns = wsum[:last] + d
        feas = (wcnt[:last] < 128) & (ns <= CAP).all(axis=1)
        if feas.any():
            load = np.where(feas, ns.max(axis=1), np.inf)
            best = int(np.argmin(load))
        else:
            nsall = wsum + d
            dcost = (np.ceil(nsall / 128) - np.ceil(wsum / 128)).sum(axis=1)
            dcost[wcnt >= 128] = np.inf
            best = int(np.argmin(dcost))
        members[best].append(nd)
        wsum[best] += d
        wcnt[best] += 1
    return [np.array(m, np.int64) for m in members]


def _repair(members, deg, CAP=256.0, iters=4000):
    """Local-search swaps to push every (window<7, type) load under CAP so the
    cross-core budget max stays at 2 chunks outside the spill window."""
    last = WPG - 1
    deg = deg.astype(np.float64)
    wsum = np.stack([deg[m].sum(0) for m in members])
    mem = [list(m) for m in members]
    for _ in range(iters):
        over = np.argwhere(wsum[:last] > CAP)
        if len(over) == 0:
            break
        w, t = over[0]
        cand = sorted(mem[w], key=lambda n: -deg[n][t])
        done = False
        for nd in cand[:20]:
            dn = deg[nd]
            for w2 in range(last):
                if w2 == w:
                    continue
                for nd2 in sorted(mem[w2], key=lambda n: deg[n][t])[:20]:
                    dn2 = deg[nd2]
                    ns_w = wsum[w] - dn + dn2
                    ns_w2 = wsum[w2] - dn2 + dn
                    if (ns_w <= CAP).all() and (ns_w2 <= CAP).all():
                        mem[w].remove(nd); mem[w].append(nd2)
                        mem[w2].remove(nd2); mem[w2].append(nd)
                        wsum[w] = ns_w; wsum[w2] = ns_w2
                        done = True
                        break
                if done:
                    break
            if done:
                break
        if not done:
            for nd in cand[:20]:
                dn = deg[nd]
                for nd2 in sorted(mem[last], key=lambda n: deg[n][t])[:40]:
                    dn2 = deg[nd2]
                    ns_w = wsum[w] - dn + dn2
                    if (ns_w <= CAP).all():
                        mem[w].remove(nd); mem[w].append(nd2)
                        mem[last].remove(nd2); mem[last].append(nd)
                        wsum[last] += dn - dn2
                        wsum[w] = ns_w
                        done = True
                        break
                if done:
                    break
        if not done:
            break
    return [np.array(m, np.int64) for m in mem]


def _prep(node_features, edge_index, edge_type, Wp, bp, Wm, bm, Wih, Whh, bih, bhh):
    """Host-side sharding/packing. Returns (meta, in_maps)."""
    x = np.asarray(node_features, np.float32)
    B = x.shape[0]
    N = B * MAXN
    GPC = B // NCORES             # graphs per core
    NB = GPC * MAXN               # nodes per core
    NWIN = GPC * WPG              # windows per core
    src = np.asarray(edge_index[0]).astype(np.int64)
    dst = np.asarray(edge_index[1]).astype(np.int64)
    et = np.asarray(edge_type).astype(np.int64)

    # per-(node, type) in-degree
    cnt = np.zeros((N, T), np.int64)
    np.add.at(cnt, (dst, et), 1)

    # balance windows within each graph -> node permutation
    old2new = np.empty(N, np.int64)
    for g in range(B):
        deg_g = cnt[g * MAXN:(g + 1) * MAXN]
        mem = _repair(_balance_graph(deg_g), deg_g)
        for w in range(WPG):
            pos = g * MAXN + w * WIN + np.arange(WIN)
            old2new[g * MAXN + mem[w]] = pos
    new2old = np.argsort(old2new)

    src_n = old2new[src]
    dst_n = old2new[dst]

    # group edges per core: key = ((gslot*WPG + w)*T + t)
    core = dst_n // NB
    rel = dst_n % NB
    win_in_core = rel // WIN      # 0..NWIN-1  (gslot*WPG + w)
    col = rel % WIN
    key = win_in_core * T + et
    NGRP = NWIN * T

    gsizes = np.zeros((NCORES, NGRP), np.int64)
    for c in range(NCORES):
        m = core == c
        gsizes[c] = np.bincount(key[m], minlength=NGRP)
    budget = np.ceil(gsizes.max(axis=0) / 128).astype(np.int64)  # chunks per group
    budget = np.maximum(budget, 1)
    ctot = int(budget.sum())
    ngg = (ctot + GSZ - 1) // GSZ      # gather groups of GSZ chunks
    ctotP = ngg * GSZ
    nslots = ctotP * 128
    gbase = np.concatenate([[0], np.cumsum(budget)])[:-1] * 128  # slot base per group

    # per-core slot arrays
    idx_maps, smat_maps = [], []
    counts_maps, xT_maps = [], []
    for c in range(NCORES):
        m = core == c
        kc, cc, sc = key[m], col[m], src_n[m]
        order = np.argsort(kc, kind="stable")
        kc, cc, sc = kc[order], cc[order], sc[order]
        # rank within group
        grp_start = np.searchsorted(kc, np.arange(NGRP), side="left")
        rank = np.arange(kc.size) - grp_start[kc]
        slot = gbase[kc] + rank
        src16 = np.zeros(nslots, np.int16)
        scol = np.full(nslots, -1, np.int64)
        src16[slot] = sc.astype(np.int16)
        scol[slot] = cc
        # idx: wrapped [16, nslots/16] replicated to 128 partitions
        idx = np.tile(src16.reshape(nslots // 16, 16).T, (8, 1)).copy()
        idx_maps.append(idx)
        # one-hot S: [ngg, 128, GSZ, 128] bf16
        smat = np.zeros((ctotP * 128, WIN), BF16)
        valid = scol >= 0
        smat[np.nonzero(valid)[0], scol[valid]] = 1
        smat = smat.reshape(ngg, GSZ, 128, WIN)
        smat = np.ascontiguousarray(smat.transpose(0, 2, 1, 3))  # [ngg,128,GSZ,128]
        smat_maps.append(smat)
        # counts (new order), [T, NB] bf16
        cslice = cnt[new2old[c * NB:(c + 1) * NB]]
        counts_maps.append(np.ascontiguousarray(cslice.T).astype(BF16))
        # xT [128, 2, NB] bf16: [p, k, node] = x[node, k*128+p]
        xs = x.reshape(N, F)[new2old[c * NB:(c + 1) * NB]]
        xp = np.zeros((NB, 2 * 128), np.float32)
        xp[:, :F] = xs
        xT = np.ascontiguousarray(xp.reshape(NB, 2, 128).transpose(2, 1, 0))
        xT_maps.append(xT.astype(BF16))

    # full permuted x as the layer-0 gather table (F padded to 256); by
    # linearity layer 0 aggregates raw x rows and the message matmul uses
    # Wm[0] @ Wp (weight folding), so no AllGather is needed for layer 0.
    xtbl = np.zeros((N, 2 * 128), np.float32)
    xtbl[:, :F] = x.reshape(N, F)[new2old]
    xtbl = xtbl.astype(BF16)

    # weights (shared across cores)
    Wp = np.asarray(Wp, np.float32); bp_ = np.asarray(bp, np.float32)
    Wm_ = np.asarray(Wm, np.float32); bm_ = np.asarray(bm, np.float32)
    Wih_ = np.asarray(Wih, np.float32); Whh_ = np.asarray(Whh, np.float32)
    bih_ = np.asarray(bih, np.float32); bhh_ = np.asarray(bhh, np.float32)

    wpT = np.zeros((128, 2, H), np.float32)          # [p, fk, h']
    wpt = Wp.T                                       # [F, H]
    wpT[:, 0, :] = wpt[0:128]
    wpT[:F - 128, 1, :] = wpt[128:F]
    wp_in = wpT.astype(BF16)
    bp_in = np.ascontiguousarray(bp_.reshape(2, 128).T)          # [128, 2]

    # fold the input projection into layer 0's message weights: layer 0
    # aggregates raw x rows, so
    #   Wm0p[t,f,e] = sum_d Wm[0,t,e,d] Wp[d,f],  bm0p[t] = Wm[0,t] @ bp + bm[0,t]
    WmIN = np.zeros((L, T, 2 * 128, H), np.float32)   # [L, T, in(padded), out]
    WmIN[1:, :, :H, :] = Wm_[1:].transpose(0, 1, 3, 2)
    WmIN[0, :, :F, :] = np.einsum('ted,df->tfe', Wm_[0], Wp)
    bm_2 = bm_.copy()
    bm_2[0] = bm_[0] + np.einsum('ted,d->te', Wm_[0], bp_)
    bm_in = bm_2.astype(BF16)                         # [L, T, H]
    wm_in = np.ascontiguousarray(                     # [L, 128, 2, T, H]
        WmIN.reshape(L, T, 2, 128, H).transpose(0, 3, 2, 1, 4)).astype(BF16)
    wih_in = np.ascontiguousarray(                    # [L, 128, 2, 3H]
        Wih_.transpose(0, 2, 1).reshape(L, 2, 128, 3 * H).transpose(0, 2, 1, 3)
    ).astype(BF16)
    whh_in = np.ascontiguousarray(
        Whh_.transpose(0, 2, 1).reshape(L, 2, 128, 3 * H).transpose(0, 2, 1, 3)
    ).astype(BF16)
    brz = bih_[:, :2 * H] + bhh_[:, :2 * H]
    brz_in = np.ascontiguousarray(brz.reshape(L, 4, 128).transpose(0, 2, 1))  # [L,128,4]
    bin_in = np.ascontiguousarray(bih_[:, 2 * H:].reshape(L, 2, 128).transpose(0, 2, 1))
    bhn_in = np.ascontiguousarray(bhh_[:, 2 * H:].reshape(L, 2, 128).transpose(0, 2, 1))
    id128 = np.eye(128, dtype=BF16)

    in_maps = []
    for c in range(NCORES):
        in_maps.append({
            "xT": xT_maps[c], "idx": idx_maps[c], "smat": smat_maps[c],
            "countsT": counts_maps[c], "xtbl": xtbl,
            "wpT": wp_in, "bp": bp_in, "wmT": wm_in, "bmT": bm_in,
            "wihT": wih_in, "whhT": whh_in,
            "brz": brz_in, "bin_": bin_in, "bhn": bhn_in, "id128": id128,
        })
    meta = dict(B=B, N=N, GPC=GPC, NB=NB, NWIN=NWIN,
                budget=budget.reshape(NWIN, T), ctot=ctot, ngg=ngg,
                new2old=new2old)
    return meta, in_maps


def _build(meta, debug=False, skip=()):
    """Build the SPMD Bass program (identical across cores)."""
    skip = frozenset(skip)
    dt = mybir.dt
    N, NB, GPC, NWIN = meta["N"], meta["NB"], meta["GPC"], meta["NWIN"]
    budget, ngg = meta["budget"], meta["ngg"]
    ctotP = ngg * GSZ
    SLOT16 = ctotP * 128 // 16

    nc = bacc.Bacc("TRN2", target_bir_lowering=False, debug=False,
                   enable_asserts=False, num_devices=NCORES)

    # ---- I/O
    xT_in = nc.dram_tensor("xT", [128, 2, NB], dt.bfloat16, kind="ExternalInput").ap()
    xtbl_in = nc.dram_tensor("xtbl", [N, 2 * 128], dt.bfloat16, kind="ExternalInput").ap()
    idx_in = nc.dram_tensor("idx", [128, SLOT16], dt.int16, kind="ExternalInput").ap()
    smat_in = nc.dram_tensor("smat", [ngg, 128, GSZ, WIN], dt.bfloat16, kind="ExternalInput").ap()
    counts_in = nc.dram_tensor("countsT", [T, NB], dt.bfloat16, kind="ExternalInput").ap()
    wp_in = nc.dram_tensor("wpT", [128, 2, H], dt.bfloat16, kind="ExternalInput").ap()
    bp_in = nc.dram_tensor("bp", [128, 2], dt.float32, kind="ExternalInput").ap()
    wm_in = nc.dram_tensor("wmT", [L, 128, 2, T, H], dt.bfloat16, kind="ExternalInput").ap()
    bm_in = nc.dram_tensor("bmT", [L, T, H], dt.bfloat16, kind="ExternalInput").ap()
    wih_in = nc.dram_tensor("wihT", [L, 128, 2, 3 * H], dt.bfloat16, kind="ExternalInput").ap()
    whh_in = nc.dram_tensor("whhT", [L, 128, 2, 3 * H], dt.bfloat16, kind="ExternalInput").ap()
    brz_in = nc.dram_tensor("brz", [L, 128, 4], dt.float32, kind="ExternalInput").ap()
    bin_in = nc.dram_tensor("bin_", [L, 128, 2], dt.float32, kind="ExternalInput").ap()
    bhn_in = nc.dram_tensor("bhn", [L, 128, 2], dt.float32, kind="ExternalInput").ap()
    id_in = nc.dram_tensor("id128", [128, 128], dt.bfloat16, kind="ExternalInput").ap()
    out_t = nc.dram_tensor("outT", [2, 128, GPC], dt.float32, kind="ExternalOutput").ap()

    groups = [list(range(NCORES))]

    with tile.TileContext(nc) as tc:
        with (
            tc.tile_pool(name="per", bufs=1) as per,       # persistent SBUF
            tc.tile_pool(name="wts", bufs=2) as wts,       # per-layer weights
            tc.tile_pool(name="gth", bufs=3) as gth,       # gather/S stream
            tc.tile_pool(name="wrk", bufs=2) as wrk,       # A/mT/staging
            tc.tile_pool(name="gru", bufs=6) as grup,      # GRU temps
            tc.tile_pool(name="ps", bufs=1, space="PSUM") as ps,
            tc.tile_pool(name="dram", bufs=2, space="DRAM") as dram,
        ):
            # persistent loads
            idx_sb = per.tile([128, SLOT16], dt.int16)
            nc.sync.dma_start(idx_sb[:], idx_in[:])
            counts_sb = per.tile([T, NB], dt.bfloat16)
            nc.sync.dma_start(counts_sb[:], counts_in[:])
            wp_sb = per.tile([128, 2, H], dt.bfloat16)
            nc.sync.dma_start(wp_sb[:], wp_in[:])
            bp_sb = per.tile([128, 2], dt.float32)
            nc.sync.dma_start(bp_sb[:], bp_in[:])
            id_sb = per.tile([128, 128], dt.bfloat16)
            nc.sync.dma_start(id_sb[:], id_in[:])
            xT_sb = per.tile([128, 2, NB], dt.bfloat16)
            nc.sync.dma_start(xT_sb[:], xT_in[:])
            hT_sb = per.tile([128, 2, NB], dt.bfloat16)
            outsb = per.tile([128, 2, GPC], dt.float32)
            nc.vector.memset(outsb[:], 0.0)
            # one shared register for every gather's num_idxs (saves a per-call
            # MOVE on the gpsimd queue)
            nidx_reg = nc.gpsimd.to_reg(GSZ * 128)

            # agin/tbl DRAM tiles per stage (after-l0, after-l1); layer 0
            # gathers straight from the xtbl input, so no stage for it.
            agins = [dram.tile([NB, H], dt.bfloat16, tag="agin", name=f"agin{i}")
                     for i in range(L - 1)]
            tbls = [dram.tile([N, H], dt.bfloat16, tag="tbl", addr_space="Shared",
                              name=f"tbl{i}") for i in range(L - 1)]

            HWPG = WPG // 2                     # windows per half-graph

            def stage_half(q, half, stage_i):
                """Transpose a half-graph's h windows to node-major and DMA into
                agins[stage_i]; fire the AllGather after the last half."""
                stg = wrk.tile([128, HWPG, H], dt.bfloat16, tag="stg", bufs=2)
                for wl in range(HWPG):
                    w = q * WPG + half * HWPG + wl
                    for hc in range(2):
                        tp = ps.tile([128, 128], dt.bfloat16, tag="tp", bufs=1)
                        nc.tensor.transpose(tp[:], hT_sb[:, hc, w * 128:(w + 1) * 128],
                                            id_sb[:])
                        nc.scalar.copy(stg[:, wl, hc * 128:(hc + 1) * 128], tp[:])
                dst_ap = agins[stage_i].rearrange("(w p) h -> p w h", p=128)
                wb = q * WPG + half * HWPG
                nc.sync.dma_start(dst_ap[:, wb:wb + HWPG, :], stg[:])
                if half == 1 and q == GPC - 1:
                    if "ag" not in skip:
                        nc.gpsimd.collective_compute(
                            "AllGather", mybir.AluOpType.bypass,
                            replica_groups=groups,
                            ins=[agins[stage_i].opt()], outs=[tbls[stage_i].opt()])
                    else:
                        nc.sync.dma_start(tbls[stage_i][0:NB], agins[stage_i][:])

            # ---- input projection: hT = Wp @ xT + bp (local h only; layer 0's
            # table is the xtbl input, so nothing to stage here)
            for s in range(NB // 512):
                for hm in range(2):
                    pm = ps.tile([128, 512], dt.float32, tag="mT", bufs=2)
                    nc.tensor.matmul(pm[:], wp_sb[:, 0, hm * 128:(hm + 1) * 128],
                                     xT_sb[:, 0, s * 512:(s + 1) * 512],
                                     start=True, stop=False)
                    nc.tensor.matmul(pm[:], wp_sb[:, 1, hm * 128:(hm + 1) * 128],
                                     xT_sb[:, 1, s * 512:(s + 1) * 512],
                                     start=False, stop=True)
                    nc.vector.tensor_scalar_add(hT_sb[:, hm, s * 512:(s + 1) * 512],
                                                pm[:], bp_sb[:, hm:hm + 1])

            for l in range(L):
                tbl = xtbl_in if l == 0 else tbls[l - 1]
                # ---- layer weights
                wm_sb = wts.tile([128, 2, T, H], dt.bfloat16, tag="wm")
                nc.sync.dma_start(wm_sb[:], wm_in[l])
                bm_sb = wts.tile([T, H], dt.bfloat16, tag="bm")
                nc.sync.dma_start(bm_sb[:], bm_in[l])
                wih_sb = wts.tile([128, 2, 3 * H], dt.bfloat16, tag="wih")
                nc.sync.dma_start(wih_sb[:], wih_in[l])
                whh_sb = wts.tile([128, 2, 3 * H], dt.bfloat16, tag="whh")
                nc.sync.dma_start(whh_sb[:], whh_in[l])
                brz_sb = wts.tile([128, 4], dt.float32, tag="brz")
                nc.sync.dma_start(brz_sb[:], brz_in[l])
                bin_sb = wts.tile([128, 2], dt.float32, tag="bin")
                nc.sync.dma_start(bin_sb[:], bin_in[l])
                bhn_sb = wts.tile([128, 2], dt.float32, tag="bhn")
                nc.sync.dma_start(bhn_sb[:], bhn_in[l])

                # ---- aggregation + message + GRU, one graph (1024 nodes) at a time
                cglob = 0          # global chunk counter (program order)
                gg_tiles = {}      # gather-group -> (G, S)

                def need(c, l=l, tbl=tbl, gg_tiles=gg_tiles):
                    gg = c // GSZ
                    while len(gg_tiles) == 0 or max(gg_tiles) < gg:
                        g_ = 0 if not gg_tiles else max(gg_tiles) + 1
                        Gt = gth.tile([128, GSZ, H], dt.bfloat16, tag="G", bufs=6,
                                      name=f"G_{l}_{g_}")
                        if "gather" not in skip:
                            nc.gpsimd.dma_gather(
                                Gt[:], tbl[:],
                                idx_sb[:, g_ * GSZ * 8:(g_ + 1) * GSZ * 8],
                                num_idxs=GSZ * 128, num_idxs_reg=nidx_reg,
                                elem_size=H)
                        else:
                            nc.sync.dma_start(
                                Gt[:],
                                tbl[0:GSZ * 128].rearrange("(c p) h -> p c h", p=128))
                        St = gth.tile([128, GSZ, WIN], dt.bfloat16, tag="S", bufs=6,
                                      name=f"S_{l}_{g_}")
                        if "sload" not in skip:
                            nc.sync.dma_start(St[:], smat_in[g_])
                        else:
                            nc.sync.dma_start(St[:], smat_in[0])
                        gg_tiles[g_] = (Gt, St)
                        if len(gg_tiles) > 4:
                            del gg_tiles[min(gg_tiles)]
                    return gg_tiles[gg], c % GSZ

                for q in range(GPC):
                    for half in range(2):
                        # per-half A with two buffers: the next half's PSUM
                        # copies need not wait for this half's message matmuls
                        # to finish reading (same total SBUF as one per-graph A)
                        A_sb = wrk.tile([128, T, 2, HWPG, WIN], dt.bfloat16,
                                        tag="A", bufs=2)
                        for wl in range(half * HWPG, (half + 1) * HWPG):
                            w = q * WPG + wl
                            for th in range(T // 2):
                                pa = ps.tile([128, 512], dt.float32, tag="agg", bufs=2)
                                for ti in range(2):
                                    t = th * 2 + ti
                                    nchunks = int(budget[w, t])
                                    for hc in range(2):
                                        off = (ti * 2 + hc) * 128
                                        for ci in range(nchunks):
                                            (Gt, St), j = need(cglob + ci)
                                            if "aggmm" in skip:
                                                continue
                                            nc.tensor.matmul(
                                                pa[:, off:off + 128],
                                                Gt[:, j, hc * 128:(hc + 1) * 128],
                                                St[:, j, :],
                                                start=(ci == 0), stop=(ci == nchunks - 1))
                                    cglob += nchunks
                                dst_ap = A_sb[:, th * 2:th * 2 + 2, :,
                                              wl - half * HWPG, :]
                                src_ap = pa.rearrange("p (t c k) -> p t c k", t=2, c=2)
                                if "aggcp" not in skip:
                                    if th % 2 == 0:
                                        nc.scalar.copy(dst_ap, src_ap)
                                    else:
                                        nc.vector.tensor_copy(dst_ap, src_ap)

                        # ---- message matmuls for this half: mT = sum_t WmT[t] @ A_t
                        mT_sb = wrk.tile([128, 2, 512], dt.bfloat16, tag="mT")
                        nbase = q * MAXN + half * 512
                        for hm in range(2):
                            pm = ps.tile([128, 512], dt.float32, tag="mT", bufs=2)
                            if "wt" not in skip:
                                nc.tensor.matmul(
                                    pm[:], bm_sb[:, hm * 128:(hm + 1) * 128],
                                    counts_sb[:, nbase:nbase + 512],
                                    start=True, stop=False)
                                for t in range(T):
                                    for hk in range(2):
                                        nc.tensor.matmul(
                                            pm[:],
                                            wm_sb[:, hk, t, hm * 128:(hm + 1) * 128],
                                            A_sb[:, t, hk, :, :],
                                            start=False, stop=(t == T - 1 and hk == 1))
                                nc.vector.tensor_copy(mT_sb[:, hm, :], pm[:])

                        # ---- GRU for this half's 512 nodes
                        if "gru" in skip:
                            continue
                        nsl = slice(nbase, nbase + 512)
                        r_sb = grup.tile([128, 2, 512], dt.float32, tag="r", bufs=2)
                        z_sb = grup.tile([128, 2, 512], dt.float32, tag="z", bufs=2)
                        for gm in range(4):
                            pg = ps.tile([128, 512], dt.float32, tag="gru", bufs=3)
                            gsl = slice(gm * 128, (gm + 1) * 128)
                            nc.tensor.matmul(pg[:], wih_sb[:, 0, gsl], mT_sb[:, 0, :],
                                             start=True, stop=False)
                            nc.tensor.matmul(pg[:], wih_sb[:, 1, gsl], mT_sb[:, 1, :],
                                             start=False, stop=False)
                            nc.tensor.matmul(pg[:], whh_sb[:, 0, gsl], hT_sb[:, 0, nsl],
                                             start=False, stop=False)
                            nc.tensor.matmul(pg[:], whh_sb[:, 1, gsl], hT_sb[:, 1, nsl],
                                             start=False, stop=True)
                            dst = r_sb[:, gm, :] if gm < 2 else z_sb[:, gm - 2, :]
                            nc.scalar.activation(dst, pg[:],
                                                 mybir.ActivationFunctionType.Sigmoid,
                                                 bias=brz_sb[:, gm:gm + 1])
                        nns, zds = [], []
                        for hc in range(2):
                            gsl = slice((4 + hc) * 128, (5 + hc) * 128)
                            ph = ps.tile([128, 512], dt.float32, tag="gru", bufs=3)
                            nc.tensor.matmul(ph[:], whh_sb[:, 0, gsl], hT_sb[:, 0, nsl],
                                             start=True, stop=False)
                            nc.tensor.matmul(ph[:], whh_sb[:, 1, gsl], hT_sb[:, 1, nsl],
                                             start=False, stop=True)
                            hnb = grup.tile([128, 512], dt.float32, tag="gt", bufs=4)
                            nc.vector.tensor_scalar_add(hnb[:], ph[:], bhn_sb[:, hc:hc + 1])
                            rhn = grup.tile([128, 512], dt.float32, tag="gt", bufs=4)
                            nc.vector.tensor_mul(rhn[:], r_sb[:, hc, :], hnb[:])
                            pi = ps.tile([128, 512], dt.float32, tag="gru", bufs=3)
                            nc.tensor.matmul(pi[:], wih_sb[:, 0, gsl], mT_sb[:, 0, :],
                                             start=True, stop=False)
                            nc.tensor.matmul(pi[:], wih_sb[:, 1, gsl], mT_sb[:, 1, :],
                                             start=False, stop=True)
                            tsum = grup.tile([128, 512], dt.float32, tag="gt", bufs=4)
                            nc.vector.tensor_add(tsum[:], pi[:], rhn[:])
                            nn = grup.tile([128, 512], dt.float32, tag="nnb", bufs=3)
                            nc.scalar.activation(nn[:], tsum[:],
                                                 mybir.ActivationFunctionType.Tanh,
                                                 bias=bin_sb[:, hc:hc + 1])
                            d_ = grup.tile([128, 512], dt.float32, tag="gt", bufs=4)
                            nc.vector.tensor_sub(d_[:], hT_sb[:, hc, nsl], nn[:])
                            zd = grup.tile([128, 512], dt.float32, tag="zdb", bufs=3)
                            nc.vector.tensor_mul(zd[:], z_sb[:, hc, :], d_[:])
                            nns.append(nn)
                            zds.append(zd)
                        # write h only after BOTH halves' matmuls consumed h_l
                        for hc in range(2):
                            if l < L - 1:
                                nc.vector.tensor_add(hT_sb[:, hc, nsl], nns[hc][:], zds[hc][:])
                            else:
                                hf = grup.tile([128, 512], dt.float32, tag="hf", bufs=2)
                                nc.vector.tensor_add(hf[:], nns[hc][:], zds[hc][:])
                                rs = grup.tile([128, 1], dt.float32, tag="rs", bufs=16)
                                nc.vector.tensor_reduce(rs[:], hf[:],
                                                        axis=mybir.AxisListType.X,
                                                        op=mybir.AluOpType.add)
                                if half == 0:
                                    nc.vector.tensor_copy(outsb[:, hc, q:q + 1], rs[:])
                                else:
                                    nc.vector.tensor_add(outsb[:, hc, q:q + 1],
                                                         outsb[:, hc, q:q + 1], rs[:])
                        # stage this half's new h for the next layer's table
                        if l < L - 1:
                            stage_half(q, half, l)
                assert cglob == int(budget.sum()), (cglob, int(budget.sum()))

            # ---- readout
            nc.sync.dma_start(out_t.rearrange("c p g -> p c g"), outsb[:])

    nc.compile()
    return nc


def kernel(**inputs):
    meta, in_maps = _prep(**inputs)
    nc = _build(meta)
    res = run_bass_kernel_spmd(nc, in_maps, core_ids=list(range(NCORES)))
    GPC = meta["GPC"]
    out = np.zeros((meta["B"], H), np.float32)
    for c in range(NCORES):
        ot = res.results[c]["outT"]          # [2, 128, GPC]
        for g in range(GPC):
            out[c * GPC + g] = np.concatenate([ot[0, :, g], ot[1, :, g]])
    return out



# revision 3
# speedup vs baseline: 1.4827x; 1.4827x over previous
"""BatchGGNNEncoder Trainium2 kernel: 8-core SPMD, dst-sharded message passing.

Full inputs in, full output out. Internally:
  - core c owns nodes [c*4096, (c+1)*4096) = graphs [4c, 4c+4) (data parallel).
  - aggregate-first GGNN layer:
        A_t[v] = sum_{e: dst=v, type=t} h[src_e]         (one-hot matmuls, PSUM)
        m      = sum_t A_t @ Wm[t].T + counts_t * bm[t]  (dense matmuls)
        h      = GRU(m, h)                               (matmuls + DVE/ACT)
  - h table (bf16, node-major) lives in DRAM, AllGathered across cores per layer;
    per-edge h[src] rows fetched with dma_gather (the kernel's critical path:
    ~8.4ns/edge of Q7 descriptor generation).
  - staging (transpose to node-major + DMA) for layer l+1's table is fused into
    layer l's per-graph GRU tail so the AllGather fires as early as possible.
  - nodes are permuted within each graph to balance (type, 128-dst-window) group
    sizes so the compiled program structure is identical on all 8 cores.
"""
import numpy as np
import ml_dtypes

import concourse.bass as bass
import concourse.bacc as bacc
import concourse.mybir as mybir
import concourse.tile as tile
from concourse.bass_utils import run_bass_kernel_spmd

BF16 = ml_dtypes.bfloat16

# problem constants (hardcoded per harness contract)
MAXN, F, H, T, L = 1024, 215, 256, 8, 3
NCORES = 8
WIN = 128                     # dst window (one-hot free width)
WPG = MAXN // WIN             # 8 windows per graph
GSZ = 8                       # chunks per dma_gather (8*128=1024 idxs; the SWDGE
                              # ring holds 64 m2s + 64 s2m pairs per engine, so
                              # 1024 idxs is the hard maximum per call)


def _balance_graph(deg):
    """Assign 1024 nodes (deg: [1024, T] type-degrees) to 8 windows of 128.
    Window WPG-1 takes the heaviest 128 nodes (the graph's excess, ~3 chunks
    per type); the remaining 896 are balanced across windows 0..WPG-2 under a
    hard 256 cap per type (2 chunks), with real slack since the heavy nodes
    are gone. Keeps cross-core max budgets at 2 for most groups."""
    tot = deg.sum(1)
    order = np.argsort(-tot, kind="stable")
    last = WPG - 1
    wsum = np.zeros((WPG, T), np.float64)
    wcnt = np.zeros(WPG, np.int64)
    members = [[] for _ in range(WPG)]
    CAP, CAP7 = 256.0, 381.0
    rest = []
    for nd in order:
        if wcnt[last] < 128 and ((wsum[last] + deg[nd]) <= CAP7).all():
            members[last].append(nd)
            wsum[last] += deg[nd]
            wcnt[last] += 1
        else:
            rest.append(nd)
    for nd in rest:
        d = deg[nd]
        ns = wsum[:last] + d
        feas = (wcnt[:last] < 128) & (ns <= CAP).all(axis=1)
        if feas.any():
            load = np.where(feas, ns.max(axis=1), np.inf)
            best = int(np.argmin(load))
        else:
            nsall = wsum + d
            dcost = (np.ceil(nsall / 128) - np.ceil(wsum / 128)).sum(axis=1)
            dcost[wcnt >= 128] = np.inf
            best = int(np.argmin(dcost))
        members[best].append(nd)
        wsum[best] += d
        wcnt[best] += 1
    return [np.array(m, np.int64) for m in members]


def _repair(members, deg, CAP=256.0, iters=4000):
    """Local-search swaps to push every (window<7, type) load under CAP so the
    cross-core budget max stays at 2 chunks outside the spill window."""
    last = WPG - 1
    deg = deg.astype(np.float64)
    wsum = np.stack([deg[m].sum(0) for m in members])
    mem = [list(m) for m in members]
    for _ in range(iters):
        over = np.argwhere(wsum[:last] > CAP)
        if len(over) == 0:
            break
        w, t = over[0]
        cand = sorted(mem[w], key=lambda n: -deg[n][t])
        done = False
        for nd in cand[:20]:
            dn = deg[nd]
            for w2 in range(last):
                if w2 == w:
                    continue
                for nd2 in sorted(mem[w2], key=lambda n: deg[n][t])[:20]:
                    dn2 = deg[nd2]
                    ns_w = wsum[w] - dn + dn2
                    ns_w2 = wsum[w2] - dn2 + dn
                    if (ns_w <= CAP).all() and (ns_w2 <= CAP).all():
                        mem[w].remove(nd); mem[w].append(nd2)
                        mem[w2].remove(nd2); mem[w2].append(nd)
                        wsum[w] = ns_w; wsum[w2] = ns_w2
                        done = True
                        break
                if done:
                    break
            if done:
                break
        if not done:
            for nd in cand[:20]:
                dn = deg[nd]
                for nd2 in sorted(mem[last], key=lambda n: deg[n][t])[:40]:
                    dn2 = deg[nd2]
                    ns_w = wsum[w] - dn + dn2
                    if (ns_w <= CAP).all():
                        mem[w].remove(nd); mem[w].append(nd2)
                        mem[last].remove(nd2); mem[last].append(nd)
                        wsum[last] += dn - dn2
                        wsum[w] = ns_w
                        done = True
                        break
                if done:
                    break
        if not done:
            break
    return [np.array(m, np.int64) for m in mem]


def _prep(node_features, edge_index, edge_type, Wp, bp, Wm, bm, Wih, Whh, bih, bhh):
    """Host-side sharding/packing. Returns (meta, in_maps)."""
    x = np.asarray(node_features, np.float32)
    B = x.shape[0]
    N = B * MAXN
    GPC = B // NCORES             # graphs per core
    NB = GPC * MAXN               # nodes per core
    NWIN = GPC * WPG              # windows per core
    src = np.asarray(edge_index[0]).astype(np.int64)
    dst = np.asarray(edge_index[1]).astype(np.int64)
    et = np.asarray(edge_type).astype(np.int64)

    # per-(node, type) in-degree
    cnt = np.zeros((N, T), np.int64)
    np.add.at(cnt, (dst, et), 1)

    # balance windows within each graph -> node permutation
    old2new = np.empty(N, np.int64)
    for g in range(B):
        deg_g = cnt[g * MAXN:(g + 1) * MAXN]
        mem = _repair(_balance_graph(deg_g), deg_g)
        for w in range(WPG):
            pos = g * MAXN + w * WIN + np.arange(WIN)
            old2new[g * MAXN + mem[w]] = pos
    new2old = np.argsort(old2new)

    src_n = old2new[src]
    dst_n = old2new[dst]

    # group edges per core: key = ((gslot*WPG + w)*T + t)
    core = dst_n // NB
    rel = dst_n % NB
    win_in_core = rel // WIN      # 0..NWIN-1  (gslot*WPG + w)
    col = rel % WIN
    key = win_in_core * T + et
    NGRP = NWIN * T

    gsizes = np.zeros((NCORES, NGRP), np.int64)
    for c in range(NCORES):
        m = core == c
        gsizes[c] = np.bincount(key[m], minlength=NGRP)
    budget = np.ceil(gsizes.max(axis=0) / 128).astype(np.int64)  # chunks per group
    budget = np.maximum(budget, 1)
    ctot = int(budget.sum())
    ngg = (ctot + GSZ - 1) // GSZ      # gather groups of GSZ chunks
    ctotP = ngg * GSZ
    nslots = ctotP * 128
    gbase = np.concatenate([[0], np.cumsum(budget)])[:-1] * 128  # slot base per group

    # per-core slot arrays
    idx_maps, smat_maps = [], []
    counts_maps, xT_maps = [], []
    for c in range(NCORES):
        m = core == c
        kc, cc, sc = key[m], col[m], src_n[m]
        order = np.argsort(kc, kind="stable")
        kc, cc, sc = kc[order], cc[order], sc[order]
        # rank within group
        grp_start = np.searchsorted(kc, np.arange(NGRP), side="left")
        rank = np.arange(kc.size) - grp_start[kc]
        slot = gbase[kc] + rank
        src16 = np.zeros(nslots, np.int16)
        scol = np.full(nslots, -1, np.int64)
        src16[slot] = sc.astype(np.int16)
        scol[slot] = cc
        # idx: wrapped [16, nslots/16] replicated to 128 partitions
        idx = np.tile(src16.reshape(nslots // 16, 16).T, (8, 1)).copy()
        idx_maps.append(idx)
        # one-hot S: [ngg, 128, GSZ, 128] bf16
        smat = np.zeros((ctotP * 128, WIN), BF16)
        valid = scol >= 0
        smat[np.nonzero(valid)[0], scol[valid]] = 1
        smat = smat.reshape(ngg, GSZ, 128, WIN)
        smat = np.ascontiguousarray(smat.transpose(0, 2, 1, 3))  # [ngg,128,GSZ,128]
        smat_maps.append(smat)
        # counts (new order), [T, NB] bf16
        cslice = cnt[new2old[c * NB:(c + 1) * NB]]
        counts_maps.append(np.ascontiguousarray(cslice.T).astype(BF16))
        # xT [128, 2, NB] bf16: [p, k, node] = x[node, k*128+p]
        xs = x.reshape(N, F)[new2old[c * NB:(c + 1) * NB]]
        xp = np.zeros((NB, 2 * 128), np.float32)
        xp[:, :F] = xs
        xT = np.ascontiguousarray(xp.reshape(NB, 2, 128).transpose(2, 1, 0))
        xT_maps.append(xT.astype(BF16))

    # full permuted x as the layer-0 gather table (F padded to 256); by
    # linearity layer 0 aggregates raw x rows and the message matmul uses
    # Wm[0] @ Wp (weight folding), so no AllGather is needed for layer 0.
    xtbl = np.zeros((N, 2 * 128), np.float32)
    xtbl[:, :F] = x.reshape(N, F)[new2old]
    xtbl = xtbl.astype(BF16)

    # weights (shared across cores)
    Wp = np.asarray(Wp, np.float32); bp_ = np.asarray(bp, np.float32)
    Wm_ = np.asarray(Wm, np.float32); bm_ = np.asarray(bm, np.float32)
    Wih_ = np.asarray(Wih, np.float32); Whh_ = np.asarray(Whh, np.float32)
    bih_ = np.asarray(bih, np.float32); bhh_ = np.asarray(bhh, np.float32)

    wpT = np.zeros((128, 2, H), np.float32)          # [p, fk, h']
    wpt = Wp.T                                       # [F, H]
    wpT[:, 0, :] = wpt[0:128]
    wpT[:F - 128, 1, :] = wpt[128:F]
    wp_in = wpT.astype(BF16)
    bp_in = np.ascontiguousarray(bp_.reshape(2, 128).T)          # [128, 2]

    # fold the input projection into layer 0's message weights: layer 0
    # aggregates raw x rows, so
    #   Wm0p[t,f,e] = sum_d Wm[0,t,e,d] Wp[d,f],  bm0p[t] = Wm[0,t] @ bp + bm[0,t]
    WmIN = np.zeros((L, T, 2 * 128, H), np.float32)   # [L, T, in(padded), out]
    WmIN[1:, :, :H, :] = Wm_[1:].transpose(0, 1, 3, 2)
    WmIN[0, :, :F, :] = np.einsum('ted,df->tfe', Wm_[0], Wp)
    bm_2 = bm_.copy()
    bm_2[0] = bm_[0] + np.einsum('ted,d->te', Wm_[0], bp_)
    bm_in = bm_2.astype(BF16)                         # [L, T, H]
    wm_in = np.ascontiguousarray(                     # [L, 128, 2, T, H]
        WmIN.reshape(L, T, 2, 128, H).transpose(0, 3, 2, 1, 4)).astype(BF16)
    wih_in = np.ascontiguousarray(                    # [L, 128, 2, 3H]
        Wih_.transpose(0, 2, 1).reshape(L, 2, 128, 3 * H).transpose(0, 2, 1, 3)
    ).astype(BF16)
    whh_in = np.ascontiguousarray(
        Whh_.transpose(0, 2, 1).reshape(L, 2, 128, 3 * H).transpose(0, 2, 1, 3)
    ).astype(BF16)
    brz = bih_[:, :2 * H] + bhh_[:, :2 * H]
    brz_in = np.ascontiguousarray(brz.reshape(L, 4, 128).transpose(0, 2, 1))  # [L,128,4]
    bin_in = np.ascontiguousarray(bih_[:, 2 * H:].reshape(L, 2, 128).transpose(0, 2, 1))
    bhn_in = np.ascontiguousarray(bhh_[:, 2 * H:].reshape(L, 2, 128).transpose(0, 2, 1))
    id128 = np.eye(128, dtype=BF16)

    in_maps = []
    for c in range(NCORES):
        in_maps.append({
            "xT": xT_maps[c], "idx": idx_maps[c], "smat": smat_maps[c],
            "countsT": counts_maps[c], "xtbl": xtbl,
            "wpT": wp_in, "bp": bp_in, "wmT": wm_in, "bmT": bm_in,
            "wihT": wih_in, "whhT": whh_in,
            "brz": brz_in, "bin_": bin_in, "bhn": bhn_in, "id128": id128,
        })
    meta = dict(B=B, N=N, GPC=GPC, NB=NB, NWIN=NWIN,
                budget=budget.reshape(NWIN, T), ctot=ctot, ngg=ngg,
                new2old=new2old)
    return meta, in_maps


def _build(meta, debug=False, skip=()):
    """Build the SPMD Bass program (identical across cores)."""
    skip = frozenset(skip)
    dt = mybir.dt
    N, NB, GPC, NWIN = meta["N"], meta["NB"], meta["GPC"], meta["NWIN"]
    budget, ngg = meta["budget"], meta["ngg"]
    ctotP = ngg * GSZ
    SLOT16 = ctotP * 128 // 16

    nc = bacc.Bacc("TRN2", target_bir_lowering=False, debug=False,
                   enable_asserts=False, num_devices=NCORES,
                   num_swdge_queues=4)

    # ---- I/O
    xT_in = nc.dram_tensor("xT", [128, 2, NB], dt.bfloat16, kind="ExternalInput").ap()
    xtbl_in = nc.dram_tensor("xtbl", [N, 2 * 128], dt.bfloat16, kind="ExternalInput").ap()
    idx_in = nc.dram_tensor("idx", [128, SLOT16], dt.int16, kind="ExternalInput").ap()
    smat_in = nc.dram_tensor("smat", [ngg, 128, GSZ, WIN], dt.bfloat16, kind="ExternalInput").ap()
    counts_in = nc.dram_tensor("countsT", [T, NB], dt.bfloat16, kind="ExternalInput").ap()
    wp_in = nc.dram_tensor("wpT", [128, 2, H], dt.bfloat16, kind="ExternalInput").ap()
    bp_in = nc.dram_tensor("bp", [128, 2], dt.float32, kind="ExternalInput").ap()
    wm_in = nc.dram_tensor("wmT", [L, 128, 2, T, H], dt.bfloat16, kind="ExternalInput").ap()
    bm_in = nc.dram_tensor("bmT", [L, T, H], dt.bfloat16, kind="ExternalInput").ap()
    wih_in = nc.dram_tensor("wihT", [L, 128, 2, 3 * H], dt.bfloat16, kind="ExternalInput").ap()
    whh_in = nc.dram_tensor("whhT", [L, 128, 2, 3 * H], dt.bfloat16, kind="ExternalInput").ap()
    brz_in = nc.dram_tensor("brz", [L, 128, 4], dt.float32, kind="ExternalInput").ap()
    bin_in = nc.dram_tensor("bin_", [L, 128, 2], dt.float32, kind="ExternalInput").ap()
    bhn_in = nc.dram_tensor("bhn", [L, 128, 2], dt.float32, kind="ExternalInput").ap()
    id_in = nc.dram_tensor("id128", [128, 128], dt.bfloat16, kind="ExternalInput").ap()
    out_t = nc.dram_tensor("outT", [2, 128, GPC], dt.float32, kind="ExternalOutput").ap()

    groups = [list(range(NCORES))]

    with tile.TileContext(nc) as tc:
        with (
            tc.tile_pool(name="per", bufs=1) as per,       # persistent SBUF
            tc.tile_pool(name="wts", bufs=2) as wts,       # per-layer weights
            tc.tile_pool(name="gth", bufs=3) as gth,       # gather/S stream
            tc.tile_pool(name="wrk", bufs=2) as wrk,       # A/mT/staging
            tc.tile_pool(name="gru", bufs=6) as grup,      # GRU temps
            tc.tile_pool(name="ps", bufs=1, space="PSUM") as ps,
            tc.tile_pool(name="dram", bufs=2, space="DRAM") as dram,
        ):
            # persistent loads
            idx_sb = per.tile([128, SLOT16], dt.int16)
            nc.sync.dma_start(idx_sb[:], idx_in[:])
            counts_sb = per.tile([T, NB], dt.bfloat16)
            nc.sync.dma_start(counts_sb[:], counts_in[:])
            wp_sb = per.tile([128, 2, H], dt.bfloat16)
            nc.sync.dma_start(wp_sb[:], wp_in[:])
            bp_sb = per.tile([128, 2], dt.float32)
            nc.sync.dma_start(bp_sb[:], bp_in[:])
            id_sb = per.tile([128, 128], dt.bfloat16)
            nc.sync.dma_start(id_sb[:], id_in[:])
            xT_sb = per.tile([128, 2, NB], dt.bfloat16)
            nc.sync.dma_start(xT_sb[:], xT_in[:])
            hT_sb = per.tile([128, 2, NB], dt.bfloat16)
            outsb = per.tile([128, 2, GPC], dt.float32)
            nc.vector.memset(outsb[:], 0.0)
            # one shared register for every gather's num_idxs (saves a per-call
            # MOVE on the gpsimd queue)
            nidx_reg = nc.gpsimd.to_reg(GSZ * 128)

            # agin/tbl DRAM tiles per stage (after-l0, after-l1); layer 0
            # gathers straight from the xtbl input, so no stage for it.
            agins = [dram.tile([NB, H], dt.bfloat16, tag="agin", name=f"agin{i}")
                     for i in range(L - 1)]
            tbls = [dram.tile([N, H], dt.bfloat16, tag="tbl", addr_space="Shared",
                              name=f"tbl{i}") for i in range(L - 1)]

            HWPG = WPG // 2                     # windows per half-graph

            def stage_half(q, half, stage_i):
                """Transpose a half-graph's h windows to node-major and DMA into
                agins[stage_i]; fire the AllGather after the last half."""
                stg = wrk.tile([128, HWPG, H], dt.bfloat16, tag="stg", bufs=2)
                for wl in range(HWPG):
                    w = q * WPG + half * HWPG + wl
                    for hc in range(2):
                        tp = ps.tile([128, 128], dt.bfloat16, tag="tp", bufs=1)
                        nc.tensor.transpose(tp[:], hT_sb[:, hc, w * 128:(w + 1) * 128],
                                            id_sb[:])
                        nc.scalar.copy(stg[:, wl, hc * 128:(hc + 1) * 128], tp[:])
                dst_ap = agins[stage_i].rearrange("(w p) h -> p w h", p=128)
                wb = q * WPG + half * HWPG
                nc.sync.dma_start(dst_ap[:, wb:wb + HWPG, :], stg[:])
                if half == 1 and q == GPC - 1:
                    if "ag" not in skip:
                        nc.gpsimd.collective_compute(
                            "AllGather", mybir.AluOpType.bypass,
                            replica_groups=groups,
                            ins=[agins[stage_i].opt()], outs=[tbls[stage_i].opt()])
                    else:
                        nc.sync.dma_start(tbls[stage_i][0:NB], agins[stage_i][:])

            # ---- input projection: hT = Wp @ xT + bp (local h only; layer 0's
            # table is the xtbl input, so nothing to stage here)
            for s in range(NB // 512):
                for hm in range(2):
                    pm = ps.tile([128, 512], dt.float32, tag="mT", bufs=2)
                    nc.tensor.matmul(pm[:], wp_sb[:, 0, hm * 128:(hm + 1) * 128],
                                     xT_sb[:, 0, s * 512:(s + 1) * 512],
                                     start=True, stop=False)
                    nc.tensor.matmul(pm[:], wp_sb[:, 1, hm * 128:(hm + 1) * 128],
                                     xT_sb[:, 1, s * 512:(s + 1) * 512],
                                     start=False, stop=True)
                    nc.vector.tensor_scalar_add(hT_sb[:, hm, s * 512:(s + 1) * 512],
                                                pm[:], bp_sb[:, hm:hm + 1])

            for l in range(L):
                tbl = xtbl_in if l == 0 else tbls[l - 1]
                # ---- layer weights
                wm_sb = wts.tile([128, 2, T, H], dt.bfloat16, tag="wm")
                nc.sync.dma_start(wm_sb[:], wm_in[l])
                bm_sb = wts.tile([T, H], dt.bfloat16, tag="bm")
                nc.sync.dma_start(bm_sb[:], bm_in[l])
                wih_sb = wts.tile([128, 2, 3 * H], dt.bfloat16, tag="wih")
                nc.sync.dma_start(wih_sb[:], wih_in[l])
                whh_sb = wts.tile([128, 2, 3 * H], dt.bfloat16, tag="whh")
                nc.sync.dma_start(whh_sb[:], whh_in[l])
                brz_sb = wts.tile([128, 4], dt.float32, tag="brz")
                nc.sync.dma_start(brz_sb[:], brz_in[l])
                bin_sb = wts.tile([128, 2], dt.float32, tag="bin")
                nc.sync.dma_start(bin_sb[:], bin_in[l])
                bhn_sb = wts.tile([128, 2], dt.float32, tag="bhn")
                nc.sync.dma_start(bhn_sb[:], bhn_in[l])

                # ---- aggregation + message + GRU, one graph (1024 nodes) at a time
                cglob = 0          # global chunk counter (program order)
                gg_tiles = {}      # gather-group -> (G, S)

                def need(c, l=l, tbl=tbl, gg_tiles=gg_tiles):
                    gg = c // GSZ
                    while len(gg_tiles) == 0 or max(gg_tiles) < gg:
                        g_ = 0 if not gg_tiles else max(gg_tiles) + 1
                        Gt = gth.tile([128, GSZ, H], dt.bfloat16, tag="G", bufs=6,
                                      name=f"G_{l}_{g_}")
                        if "gather" not in skip:
                            # round-robin the 4 SWDGE contexts: descriptor
                            # generation for up to 4 gathers proceeds in
                            # parallel (~4x Pool-engine throughput)
                            nc.gpsimd.dma_gather(
                                Gt[:], tbl[:],
                                idx_sb[:, g_ * GSZ * 8:(g_ + 1) * GSZ * 8],
                                num_idxs=GSZ * 128, num_idxs_reg=nidx_reg,
                                elem_size=H, queue_num=g_ % 4)
                        else:
                            nc.sync.dma_start(
                                Gt[:],
                                tbl[0:GSZ * 128].rearrange("(c p) h -> p c h", p=128))
                        St = gth.tile([128, GSZ, WIN], dt.bfloat16, tag="S", bufs=6,
                                      name=f"S_{l}_{g_}")
                        if "sload" not in skip:
                            nc.sync.dma_start(St[:], smat_in[g_])
                        else:
                            nc.sync.dma_start(St[:], smat_in[0])
                        gg_tiles[g_] = (Gt, St)
                        if len(gg_tiles) > 4:
                            del gg_tiles[min(gg_tiles)]
                    return gg_tiles[gg], c % GSZ

                for q in range(GPC):
                    for half in range(2):
                        # per-half A with two buffers: the next half's PSUM
                        # copies need not wait for this half's message matmuls
                        # to finish reading (same total SBUF as one per-graph A)
                        A_sb = wrk.tile([128, T, 2, HWPG, WIN], dt.bfloat16,
                                        tag="A", bufs=2)
                        for wl in range(half * HWPG, (half + 1) * HWPG):
                            w = q * WPG + wl
                            for th in range(T // 2):
                                pa = ps.tile([128, 512], dt.float32, tag="agg", bufs=2)
                                for ti in range(2):
                                    t = th * 2 + ti
                                    nchunks = int(budget[w, t])
                                    for hc in range(2):
                                        off = (ti * 2 + hc) * 128
                                        for ci in range(nchunks):
                                            (Gt, St), j = need(cglob + ci)
                                            if "aggmm" in skip:
                                                continue
                                            nc.tensor.matmul(
                                                pa[:, off:off + 128],
                                                Gt[:, j, hc * 128:(hc + 1) * 128],
                                                St[:, j, :],
                                                start=(ci == 0), stop=(ci == nchunks - 1))
                                    cglob += nchunks
                                dst_ap = A_sb[:, th * 2:th * 2 + 2, :,
                                              wl - half * HWPG, :]
                                src_ap = pa.rearrange("p (t c k) -> p t c k", t=2, c=2)
                                if "aggcp" not in skip:
                                    if th % 2 == 0:
                                        nc.scalar.copy(dst_ap, src_ap)
                                    else:
                                        nc.vector.tensor_copy(dst_ap, src_ap)

                        # ---- message matmuls for this half: mT = sum_t WmT[t] @ A_t
                        mT_sb = wrk.tile([128, 2, 512], dt.bfloat16, tag="mT")
                        nbase = q * MAXN + half * 512
                        for hm in range(2):
                            pm = ps.tile([128, 512], dt.float32, tag="mT", bufs=2)
                            if "wt" not in skip:
                                nc.tensor.matmul(
                                    pm[:], bm_sb[:, hm * 128:(hm + 1) * 128],
                                    counts_sb[:, nbase:nbase + 512],
                                    start=True, stop=False)
                                for t in range(T):
                                    for hk in range(2):
                                        nc.tensor.matmul(
                                            pm[:],
                                            wm_sb[:, hk, t, hm * 128:(hm + 1) * 128],
                                            A_sb[:, t, hk, :, :],
                                            start=False, stop=(t == T - 1 and hk == 1))
                                nc.vector.tensor_copy(mT_sb[:, hm, :], pm[:])

                        # ---- GRU for this half's 512 nodes
                        if "gru" in skip:
                            continue
                        nsl = slice(nbase, nbase + 512)
                        r_sb = grup.tile([128, 2, 512], dt.float32, tag="r", bufs=2)
                        z_sb = grup.tile([128, 2, 512], dt.float32, tag="z", bufs=2)
                        for gm in range(4):
                            pg = ps.tile([128, 512], dt.float32, tag="gru", bufs=3)
                            gsl = slice(gm * 128, (gm + 1) * 128)
                            nc.tensor.matmul(pg[:], wih_sb[:, 0, gsl], mT_sb[:, 0, :],
                                             start=True, stop=False)
                            nc.tensor.matmul(pg[:], wih_sb[:, 1, gsl], mT_sb[:, 1, :],
                                             start=False, stop=False)
                            nc.tensor.matmul(pg[:], whh_sb[:, 0, gsl], hT_sb[:, 0, nsl],
                                             start=False, stop=False)
                            nc.tensor.matmul(pg[:], whh_sb[:, 1, gsl], hT_sb[:, 1, nsl],
                                             start=False, stop=True)
                            dst = r_sb[:, gm, :] if gm < 2 else z_sb[:, gm - 2, :]
                            nc.scalar.activation(dst, pg[:],
                                                 mybir.ActivationFunctionType.Sigmoid,
                                                 bias=brz_sb[:, gm:gm + 1])
                        nns, zds = [], []
                        for hc in range(2):
                            gsl = slice((4 + hc) * 128, (5 + hc) * 128)
                            ph = ps.tile([128, 512], dt.float32, tag="gru", bufs=3)
                            nc.tensor.matmul(ph[:], whh_sb[:, 0, gsl], hT_sb[:, 0, nsl],
                                             start=True, stop=False)
                            nc.tensor.matmul(ph[:], whh_sb[:, 1, gsl], hT_sb[:, 1, nsl],
                                             start=False, stop=True)
                            hnb = grup.tile([128, 512], dt.float32, tag="gt", bufs=4)
                            nc.vector.tensor_scalar_add(hnb[:], ph[:], bhn_sb[:, hc:hc + 1])
                            rhn = grup.tile([128, 512], dt.float32, tag="gt", bufs=4)
                            nc.vector.tensor_mul(rhn[:], r_sb[:, hc, :], hnb[:])
                            pi = ps.tile([128, 512], dt.float32, tag="gru", bufs=3)
                            nc.tensor.matmul(pi[:], wih_sb[:, 0, gsl], mT_sb[:, 0, :],
                                             start=True, stop=False)
                            nc.tensor.matmul(pi[:], wih_sb[:, 1, gsl], mT_sb[:, 1, :],
                                             start=False, stop=True)
                            tsum = grup.tile([128, 512], dt.float32, tag="gt", bufs=4)
                            nc.vector.tensor_add(tsum[:], pi[:], rhn[:])
                            nn = grup.tile([128, 512], dt.float32, tag="nnb", bufs=3)
                            nc.scalar.activation(nn[:], tsum[:],
                                                 mybir.ActivationFunctionType.Tanh,
                                                 bias=bin_sb[:, hc:hc + 1])
                            d_ = grup.tile([128, 512], dt.float32, tag="gt", bufs=4)
                            nc.vector.tensor_sub(d_[:], hT_sb[:, hc, nsl], nn[:])
                            zd = grup.tile([128, 512], dt.float32, tag="zdb", bufs=3)
                            nc.vector.tensor_mul(zd[:], z_sb[:, hc, :], d_[:])
                            nns.append(nn)
                            zds.append(zd)
                        # write h only after BOTH halves' matmuls consumed h_l
                        for hc in range(2):
                            if l < L - 1:
                                nc.vector.tensor_add(hT_sb[:, hc, nsl], nns[hc][:], zds[hc][:])
                            else:
                                hf = grup.tile([128, 512], dt.float32, tag="hf", bufs=2)
                                nc.vector.tensor_add(hf[:], nns[hc][:], zds[hc][:])
                                rs = grup.tile([128, 1], dt.float32, tag="rs", bufs=16)
                                nc.vector.tensor_reduce(rs[:], hf[:],
                                                        axis=mybir.AxisListType.X,
                                                        op=mybir.AluOpType.add)
                                if half == 0:
                                    nc.vector.tensor_copy(outsb[:, hc, q:q + 1], rs[:])
                                else:
                                    nc.vector.tensor_add(outsb[:, hc, q:q + 1],
                                                         outsb[:, hc, q:q + 1], rs[:])
                        # stage this half's new h for the next layer's table
                        if l < L - 1:
                            stage_half(q, half, l)
                assert cglob == int(budget.sum()), (cglob, int(budget.sum()))

            # ---- readout
            nc.sync.dma_start(out_t.rearrange("c p g -> p c g"), outsb[:])

    nc.compile()
    return nc


def kernel(**inputs):
    meta, in_maps = _prep(**inputs)
    nc = _build(meta)
    res = run_bass_kernel_spmd(nc, in_maps, core_ids=list(range(NCORES)))
    GPC = meta["GPC"]
    out = np.zeros((meta["B"], H), np.float32)
    for c in range(NCORES):
        ot = res.results[c]["outT"]          # [2, 128, GPC]
        for g in range(GPC):
            out[c * GPC + g] = np.concatenate([ot[0, :, g], ot[1, :, g]])
    return out



# revision 23
# speedup vs baseline: 2.1200x; 1.4298x over previous
"""BatchGGNNEncoder Trainium2 kernel: 8-core SPMD, dst-sharded message passing.

Full inputs in, full output out. Internally:
  - core c owns nodes [c*4096, (c+1)*4096) = graphs [4c, 4c+4) (data parallel).
  - aggregate-first GGNN layer:
        A_t[v] = sum_{e: dst=v, type=t} h[src_e]         (one-hot matmuls, PSUM)
        m      = sum_t A_t @ Wm[t].T + counts_t * bm[t]  (dense matmuls)
        h      = GRU(m, h)                               (matmuls + DVE/ACT)
  - h table (bf16, node-major) lives in DRAM, AllGathered across cores per layer;
    per-edge h[src] rows fetched with dma_gather (the kernel's critical path:
    ~8.4ns/edge of Q7 descriptor generation).
  - staging (transpose to node-major + DMA) for layer l+1's table is fused into
    layer l's per-graph GRU tail so the AllGather fires as early as possible.
  - nodes are permuted within each graph to balance (type, 128-dst-window) group
    sizes so the compiled program structure is identical on all 8 cores.
"""
import numpy as np
import ml_dtypes

import concourse.bass as bass
import concourse.bacc as bacc
import concourse.mybir as mybir
import concourse.tile as tile
from concourse.bass_utils import run_bass_kernel_spmd

BF16 = ml_dtypes.bfloat16
F8 = ml_dtypes.float8_e4m3

# problem constants (hardcoded per harness contract)
MAXN, F, H, T, L = 1024, 215, 256, 8, 3
NCORES = 8
WIN = 128                     # dst window (one-hot free width)
WPG = MAXN // WIN             # 8 windows per graph
GSZ = 8                       # chunks per dma_gather (8*128=1024 idxs; the SWDGE
                              # ring holds 64 m2s + 64 s2m pairs per engine, so
                              # 1024 idxs is the hard maximum per call)


def _balance_graph(deg):
    """Assign 1024 nodes (deg: [1024, T] type-degrees) to 8 windows of 128.
    Window WPG-1 takes the heaviest 128 nodes (the graph's excess, ~3 chunks
    per type); the remaining 896 are balanced across windows 0..WPG-2 under a
    hard 256 cap per type (2 chunks), with real slack since the heavy nodes
    are gone. Keeps cross-core max budgets at 2 for most groups."""
    tot = deg.sum(1)
    order = np.argsort(-tot, kind="stable")
    last = WPG - 1
    wsum = np.zeros((WPG, T), np.float64)
    wcnt = np.zeros(WPG, np.int64)
    members = [[] for _ in range(WPG)]
    CAP, CAP7 = 256.0, 381.0
    rest = []
    for nd in order:
        if wcnt[last] < 128 and ((wsum[last] + deg[nd]) <= CAP7).all():
            members[last].append(nd)
            wsum[last] += deg[nd]
            wcnt[last] += 1
        else:
            rest.append(nd)
    for nd in rest:
        d = deg[nd]
        ns = wsum[:last] + d
        feas = (wcnt[:last] < 128) & (ns <= CAP).all(axis=1)
        if feas.any():
            load = np.where(feas, ns.max(axis=1), np.inf)
            best = int(np.argmin(load))
        else:
            nsall = wsum + d
            dcost = (np.ceil(nsall / 128) - np.ceil(wsum / 128)).sum(axis=1)
            dcost[wcnt >= 128] = np.inf
            best = int(np.argmin(dcost))
        members[best].append(nd)
        wsum[best] += d
        wcnt[best] += 1
    return [np.array(m, np.int64) for m in members]


def _repair(members, deg, CAP=256.0, iters=4000):
    """Local-search swaps to push every (window<7, type) load under CAP so the
    cross-core budget max stays at 2 chunks outside the spill window."""
    last = WPG - 1
    deg = deg.astype(np.float64)
    wsum = np.stack([deg[m].sum(0) for m in members])
    mem = [list(m) for m in members]
    for _ in range(iters):
        over = np.argwhere(wsum[:last] > CAP)
        if len(over) == 0:
            break
        w, t = over[0]
        cand = sorted(mem[w], key=lambda n: -deg[n][t])
        done = False
        for nd in cand[:20]:
            dn = deg[nd]
            for w2 in range(last):
                if w2 == w:
                    continue
                for nd2 in sorted(mem[w2], key=lambda n: deg[n][t])[:20]:
                    dn2 = deg[nd2]
                    ns_w = wsum[w] - dn + dn2
                    ns_w2 = wsum[w2] - dn2 + dn
                    if (ns_w <= CAP).all() and (ns_w2 <= CAP).all():
                        mem[w].remove(nd); mem[w].append(nd2)
                        mem[w2].remove(nd2); mem[w2].append(nd)
                        wsum[w] = ns_w; wsum[w2] = ns_w2
                        done = True
                        break
                if done:
                    break
            if done:
                break
        if not done:
            for nd in cand[:20]:
                dn = deg[nd]
                for nd2 in sorted(mem[last], key=lambda n: deg[n][t])[:40]:
                    dn2 = deg[nd2]
                    ns_w = wsum[w] - dn + dn2
                    if (ns_w <= CAP).all():
                        mem[w].remove(nd); mem[w].append(nd2)
                        mem[last].remove(nd2); mem[last].append(nd)
                        wsum[last] += dn - dn2
                        wsum[w] = ns_w
                        done = True
                        break
                if done:
                    break
        if not done:
            break
    return [np.array(m, np.int64) for m in mem]


def _prep(node_features, edge_index, edge_type, Wp, bp, Wm, bm, Wih, Whh, bih, bhh):
    """Host-side sharding/packing. Returns (meta, in_maps)."""
    x = np.asarray(node_features, np.float32)
    B = x.shape[0]
    N = B * MAXN
    GPC = B // NCORES             # graphs per core
    NB = GPC * MAXN               # nodes per core
    NWIN = GPC * WPG              # windows per core
    src = np.asarray(edge_index[0]).astype(np.int64)
    dst = np.asarray(edge_index[1]).astype(np.int64)
    et = np.asarray(edge_type).astype(np.int64)

    # per-(node, type) in-degree
    cnt = np.zeros((N, T), np.int64)
    np.add.at(cnt, (dst, et), 1)

    # balance windows within each graph -> node permutation
    old2new = np.empty(N, np.int64)
    for g in range(B):
        deg_g = cnt[g * MAXN:(g + 1) * MAXN]
        mem = _repair(_balance_graph(deg_g), deg_g)
        for w in range(WPG):
            pos = g * MAXN + w * WIN + np.arange(WIN)
            old2new[g * MAXN + mem[w]] = pos
    new2old = np.argsort(old2new)

    src_n = old2new[src]
    dst_n = old2new[dst]

    # group edges per core: key = ((gslot*WPG + w)*T + t)
    core = dst_n // NB
    rel = dst_n % NB
    win_in_core = rel // WIN      # 0..NWIN-1  (gslot*WPG + w)
    col = rel % WIN
    key = win_in_core * T + et
    NGRP = NWIN * T

    gsizes = np.zeros((NCORES, NGRP), np.int64)
    for c in range(NCORES):
        m = core == c
        gsizes[c] = np.bincount(key[m], minlength=NGRP)
    budget = np.ceil(gsizes.max(axis=0) / 128).astype(np.int64)  # chunks per group
    budget = np.maximum(budget, 1)
    ctot = int(budget.sum())
    ngg = (ctot + GSZ - 1) // GSZ      # gather groups of GSZ chunks
    ctotP = ngg * GSZ
    nslots = ctotP * 128
    gbase = np.concatenate([[0], np.cumsum(budget)])[:-1] * 128  # slot base per group

    # per-core slot arrays
    idx_maps, smat_maps = [], []
    counts_maps, xT_maps = [], []
    for c in range(NCORES):
        m = core == c
        kc, cc, sc = key[m], col[m], src_n[m]
        order = np.argsort(kc, kind="stable")
        kc, cc, sc = kc[order], cc[order], sc[order]
        # rank within group
        grp_start = np.searchsorted(kc, np.arange(NGRP), side="left")
        rank = np.arange(kc.size) - grp_start[kc]
        slot = gbase[kc] + rank
        src16 = np.zeros(nslots, np.int16)
        scol = np.full(nslots, -1, np.int64)
        src16[slot] = sc.astype(np.int16)
        scol[slot] = cc
        # idx: wrapped [16, nslots/16] replicated to 128 partitions
        idx = np.tile(src16.reshape(nslots // 16, 16).T, (8, 1)).copy()
        idx_maps.append(idx)
        # one-hot S: [ngg, 128, GSZ, 128] fp8 (0/1 exact)
        smat = np.zeros((ctotP * 128, WIN), F8)
        valid = scol >= 0
        smat[np.nonzero(valid)[0], scol[valid]] = 1
        smat = smat.reshape(ngg, GSZ, 128, WIN)
        smat = np.ascontiguousarray(smat.transpose(0, 2, 1, 3))  # [ngg,128,GSZ,128]
        smat_maps.append(smat)
        # counts (new order), [T, NB] bf16
        cslice = cnt[new2old[c * NB:(c + 1) * NB]]
        counts_maps.append(np.ascontiguousarray(cslice.T).astype(BF16))
        # xT [128, 2, NB] bf16: [p, k, node] = x[node, k*128+p]
        xs = x.reshape(N, F)[new2old[c * NB:(c + 1) * NB]]
        xp = np.zeros((NB, 2 * 128), np.float32)
        xp[:, :F] = xs
        xT = np.ascontiguousarray(xp.reshape(NB, 2, 128).transpose(2, 1, 0))
        xT_maps.append(xT.astype(BF16))

    # full permuted x as the layer-0 gather table (F padded to 256); by
    # linearity layer 0 aggregates raw x rows and the message matmul uses
    # Wm[0] @ Wp (weight folding), so no AllGather is needed for layer 0.
    # fp8: gathered-row quantization noise is averaged out by the 2048-wide
    # message contraction (~0.1% effect on m), so the h/x tables, S one-hots
    # and aggregation matmuls all run in fp8e4m3.
    xtbl = np.zeros((N, 2 * 128), np.float32)
    xtbl[:, :F] = x.reshape(N, F)[new2old]
    xtbl = xtbl.astype(F8)

    # weights (shared across cores)
    Wp = np.asarray(Wp, np.float32); bp_ = np.asarray(bp, np.float32)
    Wm_ = np.asarray(Wm, np.float32); bm_ = np.asarray(bm, np.float32)
    Wih_ = np.asarray(Wih, np.float32); Whh_ = np.asarray(Whh, np.float32)
    bih_ = np.asarray(bih, np.float32); bhh_ = np.asarray(bhh, np.float32)

    wpT = np.zeros((128, 2, H), np.float32)          # [p, fk, h']
    wpt = Wp.T                                       # [F, H]
    wpT[:, 0, :] = wpt[0:128]
    wpT[:F - 128, 1, :] = wpt[128:F]
    wp_in = wpT.astype(BF16)
    bp_in = np.ascontiguousarray(bp_.reshape(2, 128).T)          # [128, 2]

    # fold the input projection into layer 0's message weights: layer 0
    # aggregates raw x rows, so
    #   Wm0p[t,f,e] = sum_d Wm[0,t,e,d] Wp[d,f],  bm0p[t] = Wm[0,t] @ bp + bm[0,t]
    WmIN = np.zeros((L, T, 2 * 128, H), np.float32)   # [L, T, in(padded), out]
    WmIN[1:, :, :H, :] = Wm_[1:].transpose(0, 1, 3, 2)
    WmIN[0, :, :F, :] = np.einsum('ted,df->tfe', Wm_[0], Wp)
    bm_2 = bm_.copy()
    bm_2[0] = bm_[0] + np.einsum('ted,d->te', Wm_[0], bp_)
    bm_in = bm_2.astype(BF16)                         # [L, T, H]
    wm_in = np.ascontiguousarray(                     # [L, 128, 2, T, H]
        WmIN.reshape(L, T, 2, 128, H).transpose(0, 3, 2, 1, 4)).astype(BF16)
    wih_in = np.ascontiguousarray(                    # [L, 128, 2, 3H]
        Wih_.transpose(0, 2, 1).reshape(L, 2, 128, 3 * H).transpose(0, 2, 1, 3)
    ).astype(BF16)
    whh_in = np.ascontiguousarray(
        Whh_.transpose(0, 2, 1).reshape(L, 2, 128, 3 * H).transpose(0, 2, 1, 3)
    ).astype(BF16)
    brz = bih_[:, :2 * H] + bhh_[:, :2 * H]
    brz_in = np.ascontiguousarray(brz.reshape(L, 4, 128).transpose(0, 2, 1))  # [L,128,4]
    bin_in = np.ascontiguousarray(bih_[:, 2 * H:].reshape(L, 2, 128).transpose(0, 2, 1))
    bhn_in = np.ascontiguousarray(bhh_[:, 2 * H:].reshape(L, 2, 128).transpose(0, 2, 1))
    id128 = np.eye(128, dtype=BF16)

    in_maps = []
    for c in range(NCORES):
        in_maps.append({
            "xT": xT_maps[c], "idx": idx_maps[c], "smat": smat_maps[c],
            "countsT": counts_maps[c], "xtbl": xtbl,
            "wpT": wp_in, "bp": bp_in, "wmT": wm_in, "bmT": bm_in,
            "wihT": wih_in, "whhT": whh_in,
            "brz": brz_in, "bin_": bin_in, "bhn": bhn_in, "id128": id128,
        })
    meta = dict(B=B, N=N, GPC=GPC, NB=NB, NWIN=NWIN,
                budget=budget.reshape(NWIN, T), ctot=ctot, ngg=ngg,
                new2old=new2old)
    return meta, in_maps


def _build(meta, debug=False, skip=()):
    """Build the SPMD Bass program (identical across cores)."""
    skip = frozenset(skip)
    dt = mybir.dt
    N, NB, GPC, NWIN = meta["N"], meta["NB"], meta["GPC"], meta["NWIN"]
    budget, ngg = meta["budget"], meta["ngg"]
    ctotP = ngg * GSZ
    SLOT16 = ctotP * 128 // 16

    nc = bacc.Bacc("TRN2", target_bir_lowering=False, debug=False,
                   enable_asserts=False, num_devices=NCORES,
                   num_swdge_queues=4)

    # ---- I/O
    xT_in = nc.dram_tensor("xT", [128, 2, NB], dt.bfloat16, kind="ExternalInput").ap()
    xtbl_in = nc.dram_tensor("xtbl", [N, 2 * 128], dt.float8e4, kind="ExternalInput").ap()
    idx_in = nc.dram_tensor("idx", [128, SLOT16], dt.int16, kind="ExternalInput").ap()
    smat_in = nc.dram_tensor("smat", [ngg, 128, GSZ, WIN], dt.float8e4, kind="ExternalInput").ap()
    counts_in = nc.dram_tensor("countsT", [T, NB], dt.bfloat16, kind="ExternalInput").ap()
    wp_in = nc.dram_tensor("wpT", [128, 2, H], dt.bfloat16, kind="ExternalInput").ap()
    bp_in = nc.dram_tensor("bp", [128, 2], dt.float32, kind="ExternalInput").ap()
    wm_in = nc.dram_tensor("wmT", [L, 128, 2, T, H], dt.bfloat16, kind="ExternalInput").ap()
    bm_in = nc.dram_tensor("bmT", [L, T, H], dt.bfloat16, kind="ExternalInput").ap()
    wih_in = nc.dram_tensor("wihT", [L, 128, 2, 3 * H], dt.bfloat16, kind="ExternalInput").ap()
    whh_in = nc.dram_tensor("whhT", [L, 128, 2, 3 * H], dt.bfloat16, kind="ExternalInput").ap()
    brz_in = nc.dram_tensor("brz", [L, 128, 4], dt.float32, kind="ExternalInput").ap()
    bin_in = nc.dram_tensor("bin_", [L, 128, 2], dt.float32, kind="ExternalInput").ap()
    bhn_in = nc.dram_tensor("bhn", [L, 128, 2], dt.float32, kind="ExternalInput").ap()
    id_in = nc.dram_tensor("id128", [128, 128], dt.bfloat16, kind="ExternalInput").ap()
    out_t = nc.dram_tensor("outT", [2, 128, GPC], dt.float32, kind="ExternalOutput").ap()

    groups = [list(range(NCORES))]

    with tile.TileContext(nc) as tc:
        with (
            tc.tile_pool(name="per", bufs=1) as per,       # persistent SBUF
            tc.tile_pool(name="wts", bufs=2) as wts,       # per-layer weights
            tc.tile_pool(name="gth", bufs=3) as gth,       # gather/S stream
            tc.tile_pool(name="wrk", bufs=2) as wrk,       # A/mT/staging
            tc.tile_pool(name="gru", bufs=6) as grup,      # GRU temps
            tc.tile_pool(name="ps", bufs=1, space="PSUM") as ps,
            tc.tile_pool(name="dram", bufs=2, space="DRAM") as dram,
        ):
            # persistent loads
            idx_sb = per.tile([128, SLOT16], dt.int16)
            nc.sync.dma_start(idx_sb[:], idx_in[:])
            counts_sb = per.tile([T, NB], dt.bfloat16)
            nc.sync.dma_start(counts_sb[:], counts_in[:])
            wp_sb = per.tile([128, 2, H], dt.bfloat16)
            nc.sync.dma_start(wp_sb[:], wp_in[:])
            bp_sb = per.tile([128, 2], dt.float32)
            nc.sync.dma_start(bp_sb[:], bp_in[:])
            id_sb = per.tile([128, 128], dt.bfloat16)
            nc.sync.dma_start(id_sb[:], id_in[:])
            xT_sb = per.tile([128, 2, NB], dt.bfloat16)
            nc.sync.dma_start(xT_sb[:], xT_in[:])
            hT_sb = per.tile([128, 2, NB], dt.bfloat16)
            outsb = per.tile([128, 2, GPC], dt.float32)
            nc.vector.memset(outsb[:], 0.0)
            # one shared register for every gather's num_idxs (saves a per-call
            # MOVE on the gpsimd queue)
            nidx_reg = nc.gpsimd.to_reg(GSZ * 128)

            # agin/tbl DRAM tiles per stage (after-l0, after-l1); layer 0
            # gathers straight from the xtbl input, so no stage for it.
            agins = [dram.tile([NB, H], dt.float8e4, tag="agin", name=f"agin{i}")
                     for i in range(L - 1)]
            tbls = [dram.tile([N, H], dt.float8e4, tag="tbl", addr_space="Shared",
                              name=f"tbl{i}") for i in range(L - 1)]

            HWPG = WPG // 2                     # windows per half-graph

            def stage_half(q, half, stage_i):
                """Transpose a half-graph's h windows to node-major and DMA into
                agins[stage_i]; fire graph q's AllGather after its last half so
                graphs 0..GPC-2's exchanges overlap the layer's gather stream
                and only graph GPC-1's sits at the layer boundary."""
                stg = wrk.tile([128, HWPG, H], dt.float8e4, tag="stg", bufs=2)
                for wl in range(HWPG):
                    w = q * WPG + half * HWPG + wl
                    for hc in range(2):
                        tp = ps.tile([128, 128], dt.bfloat16, tag="tp", bufs=1)
                        nc.tensor.transpose(tp[:], hT_sb[:, hc, w * 128:(w + 1) * 128],
                                            id_sb[:])
                        nc.scalar.copy(stg[:, wl, hc * 128:(hc + 1) * 128], tp[:])
                dst_ap = agins[stage_i].rearrange("(w p) h -> p w h", p=128)
                wb = q * WPG + half * HWPG
                nc.sync.dma_start(dst_ap[:, wb:wb + HWPG, :], stg[:])
                if half == 1 and q == GPC - 1:
                    if "ag" not in skip:
                        nc.gpsimd.collective_compute(
                            "AllGather", mybir.AluOpType.bypass,
                            replica_groups=groups,
                            ins=[agins[stage_i].opt()], outs=[tbls[stage_i].opt()])
                    else:
                        nc.sync.dma_start(tbls[stage_i][0:NB], agins[stage_i][:])

            # ---- input projection: hT = Wp @ xT + bp (local h only; layer 0's
            # table is the xtbl input, so nothing to stage here)
            for s in range(NB // 512):
                for hm in range(2):
                    pm = ps.tile([128, 512], dt.float32, tag="mT", bufs=2)
                    nc.tensor.matmul(pm[:], wp_sb[:, 0, hm * 128:(hm + 1) * 128],
                                     xT_sb[:, 0, s * 512:(s + 1) * 512],
                                     start=True, stop=False)
                    nc.tensor.matmul(pm[:], wp_sb[:, 1, hm * 128:(hm + 1) * 128],
                                     xT_sb[:, 1, s * 512:(s + 1) * 512],
                                     start=False, stop=True)
                    nc.vector.tensor_scalar_add(hT_sb[:, hm, s * 512:(s + 1) * 512],
                                                pm[:], bp_sb[:, hm:hm + 1])

            for l in range(L):
                tbl = xtbl_in if l == 0 else tbls[l - 1]
                # ---- layer weights
                wm_sb = wts.tile([128, 2, T, H], dt.bfloat16, tag="wm")
                nc.sync.dma_start(wm_sb[:], wm_in[l])
                bm_sb = wts.tile([T, H], dt.bfloat16, tag="bm")
                nc.sync.dma_start(bm_sb[:], bm_in[l])
                wih_sb = wts.tile([128, 2, 3 * H], dt.bfloat16, tag="wih")
                nc.sync.dma_start(wih_sb[:], wih_in[l])
                whh_sb = wts.tile([128, 2, 3 * H], dt.bfloat16, tag="whh")
                nc.sync.dma_start(whh_sb[:], whh_in[l])
                brz_sb = wts.tile([128, 4], dt.float32, tag="brz")
                nc.sync.dma_start(brz_sb[:], brz_in[l])
                bin_sb = wts.tile([128, 2], dt.float32, tag="bin")
                nc.sync.dma_start(bin_sb[:], bin_in[l])
                bhn_sb = wts.tile([128, 2], dt.float32, tag="bhn")
                nc.sync.dma_start(bhn_sb[:], bhn_in[l])

                # ---- aggregation + message + GRU, one graph (1024 nodes) at a time
                cglob = 0          # global chunk counter (program order)
                gg_tiles = {}      # gather-group -> (G, S)

                def need(c, l=l, tbl=tbl, gg_tiles=gg_tiles):
                    gg = c // GSZ
                    while len(gg_tiles) == 0 or max(gg_tiles) < gg:
                        g_ = 0 if not gg_tiles else max(gg_tiles) + 1
                        Gt = gth.tile([128, GSZ, H], dt.float8e4, tag="G", bufs=8,
                                      name=f"G_{l}_{g_}")
                        if "gather" not in skip:
                            # round-robin the 4 SWDGE contexts: descriptor
                            # generation for up to 4 gathers proceeds in
                            # parallel (~4x Pool-engine throughput)
                            nc.gpsimd.dma_gather(
                                Gt[:], tbl[:],
                                idx_sb[:, g_ * GSZ * 8:(g_ + 1) * GSZ * 8],
                                num_idxs=GSZ * 128, num_idxs_reg=nidx_reg,
                                elem_size=H, queue_num=g_ % 4)
                        else:
                            nc.sync.dma_start(
                                Gt[:],
                                tbl[0:GSZ * 128].rearrange("(c p) h -> p c h", p=128))
                        St = gth.tile([128, GSZ, WIN], dt.float8e4, tag="S", bufs=8,
                                      name=f"S_{l}_{g_}")
                        if "sload" not in skip:
                            nc.sync.dma_start(St[:], smat_in[g_])
                        else:
                            nc.sync.dma_start(St[:], smat_in[0])
                        gg_tiles[g_] = (Gt, St)
                        if len(gg_tiles) > 6:
                            del gg_tiles[min(gg_tiles)]
                    return gg_tiles[gg], c % GSZ

                for q in range(GPC):
                    for half in range(2):
                        # per-half A with two buffers: the next half's PSUM
                        # copies need not wait for this half's message matmuls
                        # to finish reading (same total SBUF as one per-graph A)
                        A_sb = wrk.tile([128, T, 2, HWPG, WIN], dt.bfloat16,
                                        tag="A", bufs=2)
                        for wl in range(half * HWPG, (half + 1) * HWPG):
                            w = q * WPG + wl
                            for th in range(T // 2):
                                pa = ps.tile([128, 512], dt.float32, tag="agg", bufs=2)
                                for ti in range(2):
                                    t = th * 2 + ti
                                    nchunks = int(budget[w, t])
                                    for hc in range(2):
                                        off = (ti * 2 + hc) * 128
                                        for ci in range(nchunks):
                                            (Gt, St), j = need(cglob + ci)
                                            if "aggmm" in skip:
                                                continue
                                            nc.tensor.matmul(
                                                pa[:, off:off + 128],
                                                Gt[:, j, hc * 128:(hc + 1) * 128],
                                                St[:, j, :],
                                                start=(ci == 0), stop=(ci == nchunks - 1))
                                    cglob += nchunks
                                dst_ap = A_sb[:, th * 2:th * 2 + 2, :,
                                              wl - half * HWPG, :]
                                src_ap = pa.rearrange("p (t c k) -> p t c k", t=2, c=2)
                                if "aggcp" not in skip:
                                    if th % 2 == 0:
                                        nc.scalar.copy(dst_ap, src_ap)
                                    else:
                                        nc.vector.tensor_copy(dst_ap, src_ap)

                        # ---- message matmuls for this half: mT = sum_t WmT[t] @ A_t
                        mT_sb = wrk.tile([128, 2, 512], dt.bfloat16, tag="mT")
                        nbase = q * MAXN + half * 512
                        for hm in range(2):
                            pm = ps.tile([128, 512], dt.float32, tag="mT", bufs=2)
                            if "wt" not in skip:
                                nc.tensor.matmul(
                                    pm[:], bm_sb[:, hm * 128:(hm + 1) * 128],
                                    counts_sb[:, nbase:nbase + 512],
                                    start=True, stop=False)
                                for t in range(T):
                                    for hk in range(2):
                                        nc.tensor.matmul(
                                            pm[:],
                                            wm_sb[:, hk, t, hm * 128:(hm + 1) * 128],
                                            A_sb[:, t, hk, :, :],
                                            start=False, stop=(t == T - 1 and hk == 1))
                                nc.vector.tensor_copy(mT_sb[:, hm, :], pm[:])

                        # ---- GRU for this half's 512 nodes
                        if "gru" in skip:
                            continue
                        nsl = slice(nbase, nbase + 512)
                        r_sb = grup.tile([128, 2, 512], dt.float32, tag="r", bufs=2)
                        z_sb = grup.tile([128, 2, 512], dt.float32, tag="z", bufs=2)
                        for gm in range(4):
                            pg = ps.tile([128, 512], dt.float32, tag="gru", bufs=3)
                            gsl = slice(gm * 128, (gm + 1) * 128)
                            nc.tensor.matmul(pg[:], wih_sb[:, 0, gsl], mT_sb[:, 0, :],
                                             start=True, stop=False)
                            nc.tensor.matmul(pg[:], wih_sb[:, 1, gsl], mT_sb[:, 1, :],
                                             start=False, stop=False)
                            nc.tensor.matmul(pg[:], whh_sb[:, 0, gsl], hT_sb[:, 0, nsl],
                                             start=False, stop=False)
                            nc.tensor.matmul(pg[:], whh_sb[:, 1, gsl], hT_sb[:, 1, nsl],
                                             start=False, stop=True)
                            dst = r_sb[:, gm, :] if gm < 2 else z_sb[:, gm - 2, :]
                            nc.scalar.activation(dst, pg[:],
                                                 mybir.ActivationFunctionType.Sigmoid,
                                                 bias=brz_sb[:, gm:gm + 1])
                        nns, zds = [], []
                        for hc in range(2):
                            gsl = slice((4 + hc) * 128, (5 + hc) * 128)
                            ph = ps.tile([128, 512], dt.float32, tag="gru", bufs=3)
                            nc.tensor.matmul(ph[:], whh_sb[:, 0, gsl], hT_sb[:, 0, nsl],
                                             start=True, stop=False)
                            nc.tensor.matmul(ph[:], whh_sb[:, 1, gsl], hT_sb[:, 1, nsl],
                                             start=False, stop=True)
                            hnb = grup.tile([128, 512], dt.float32, tag="gt", bufs=4)
                            nc.vector.tensor_scalar_add(hnb[:], ph[:], bhn_sb[:, hc:hc + 1])
                            rhn = grup.tile([128, 512], dt.float32, tag="gt", bufs=4)
                            nc.vector.tensor_mul(rhn[:], r_sb[:, hc, :], hnb[:])
                            pi = ps.tile([128, 512], dt.float32, tag="gru", bufs=3)
                            nc.tensor.matmul(pi[:], wih_sb[:, 0, gsl], mT_sb[:, 0, :],
                                             start=True, stop=False)
                            nc.tensor.matmul(pi[:], wih_sb[:, 1, gsl], mT_sb[:, 1, :],
                                             start=False, stop=True)
                            tsum = grup.tile([128, 512], dt.float32, tag="gt", bufs=4)
                            nc.vector.tensor_add(tsum[:], pi[:], rhn[:])
                            nn = grup.tile([128, 512], dt.float32, tag="nnb", bufs=3)
                            nc.scalar.activation(nn[:], tsum[:],
                                                 mybir.ActivationFunctionType.Tanh,
                                                 bias=bin_sb[:, hc:hc + 1])
                            d_ = grup.tile([128, 512], dt.float32, tag="gt", bufs=4)
                            nc.vector.tensor_sub(d_[:], hT_sb[:, hc, nsl], nn[:])
                            zd = grup.tile([128, 512], dt.float32, tag="zdb", bufs=3)
                            nc.vector.tensor_mul(zd[:], z_sb[:, hc, :], d_[:])
                            nns.append(nn)
                            zds.append(zd)
                        # write h only after BOTH halves' matmuls consumed h_l
                        for hc in range(2):
                            if l < L - 1:
                                nc.vector.tensor_add(hT_sb[:, hc, nsl], nns[hc][:], zds[hc][:])
                            else:
                                hf = grup.tile([128, 512], dt.float32, tag="hf", bufs=2)
                                nc.vector.tensor_add(hf[:], nns[hc][:], zds[hc][:])
                                rs = grup.tile([128, 1], dt.float32, tag="rs", bufs=16)
                                nc.vector.tensor_reduce(rs[:], hf[:],
                                                        axis=mybir.AxisListType.X,
                                                        op=mybir.AluOpType.add)
                                if half == 0:
                                    nc.vector.tensor_copy(outsb[:, hc, q:q + 1], rs[:])
                                else:
                                    nc.vector.tensor_add(outsb[:, hc, q:q + 1],
                                                         outsb[:, hc, q:q + 1], rs[:])
                        # stage this half's new h for the next layer's table
                        if l < L - 1:
                            stage_half(q, half, l)
                assert cglob == int(budget.sum()), (cglob, int(budget.sum()))

            # ---- readout
            nc.sync.dma_start(out_t.rearrange("c p g -> p c g"), outsb[:])

    nc.compile()
    return nc


def kernel(**inputs):
    meta, in_maps = _prep(**inputs)
    nc = _build(meta)
    res = run_bass_kernel_spmd(nc, in_maps, core_ids=list(range(NCORES)))
    GPC = meta["GPC"]
    out = np.zeros((meta["B"], H), np.float32)
    for c in range(NCORES):
        ot = res.results[c]["outT"]          # [2, 128, GPC]
        for g in range(GPC):
            out[c * GPC + g] = np.concatenate([ot[0, :, g], ot[1, :, g]])
    return out



# revision 27
# speedup vs baseline: 2.1569x; 1.0174x over previous
"""BatchGGNNEncoder Trainium2 kernel: 8-core SPMD, dst-sharded message passing.

Full inputs in, full output out. Internally:
  - core c owns nodes [c*4096, (c+1)*4096) = graphs [4c, 4c+4) (data parallel).
  - aggregate-first GGNN layer:
        A_t[v] = sum_{e: dst=v, type=t} h[src_e]         (one-hot matmuls, PSUM)
        m      = sum_t A_t @ Wm[t].T + counts_t * bm[t]  (dense matmuls)
        h      = GRU(m, h)                               (matmuls + DVE/ACT)
  - h table (bf16, node-major) lives in DRAM, AllGathered across cores per layer;
    per-edge h[src] rows fetched with dma_gather (the kernel's critical path:
    ~8.4ns/edge of Q7 descriptor generation).
  - staging (transpose to node-major + DMA) for layer l+1's table is fused into
    layer l's per-graph GRU tail so the AllGather fires as early as possible.
  - nodes are permuted within each graph to balance (type, 128-dst-window) group
    sizes so the compiled program structure is identical on all 8 cores.
"""
import numpy as np
import ml_dtypes

import concourse.bass as bass
import concourse.bacc as bacc
import concourse.mybir as mybir
import concourse.tile as tile
from concourse.bass_utils import run_bass_kernel_spmd

BF16 = ml_dtypes.bfloat16
F8 = ml_dtypes.float8_e4m3

# problem constants (hardcoded per harness contract)
MAXN, F, H, T, L = 1024, 215, 256, 8, 3
NCORES = 8
WIN = 128                     # dst window (one-hot free width)
WPG = MAXN // WIN             # 8 windows per graph
GSZ = 8                       # chunks per dma_gather (8*128=1024 idxs; the SWDGE
                              # ring holds 64 m2s + 64 s2m pairs per engine, so
                              # 1024 idxs is the hard maximum per call)


def _balance_graph(deg):
    """Assign 1024 nodes (deg: [1024, T] type-degrees) to 8 windows of 128.
    Window WPG-1 takes the heaviest 128 nodes (the graph's excess, ~3 chunks
    per type); the remaining 896 are balanced across windows 0..WPG-2 under a
    hard 256 cap per type (2 chunks), with real slack since the heavy nodes
    are gone. Keeps cross-core max budgets at 2 for most groups."""
    tot = deg.sum(1)
    order = np.argsort(-tot, kind="stable")
    last = WPG - 1
    wsum = np.zeros((WPG, T), np.float64)
    wcnt = np.zeros(WPG, np.int64)
    members = [[] for _ in range(WPG)]
    CAP, CAP7 = 256.0, 381.0
    rest = []
    for nd in order:
        if wcnt[last] < 128 and ((wsum[last] + deg[nd]) <= CAP7).all():
            members[last].append(nd)
            wsum[last] += deg[nd]
            wcnt[last] += 1
        else:
            rest.append(nd)
    for nd in rest:
        d = deg[nd]
        ns = wsum[:last] + d
        feas = (wcnt[:last] < 128) & (ns <= CAP).all(axis=1)
        if feas.any():
            load = np.where(feas, ns.max(axis=1), np.inf)
            best = int(np.argmin(load))
        else:
            nsall = wsum + d
            dcost = (np.ceil(nsall / 128) - np.ceil(wsum / 128)).sum(axis=1)
            dcost[wcnt >= 128] = np.inf
            best = int(np.argmin(dcost))
        members[best].append(nd)
        wsum[best] += d
        wcnt[best] += 1
    return [np.array(m, np.int64) for m in members]


def _repair(members, deg, CAP=256.0, iters=4000):
    """Local-search swaps to push every (window<7, type) load under CAP so the
    cross-core budget max stays at 2 chunks outside the spill window."""
    last = WPG - 1
    deg = deg.astype(np.float64)
    wsum = np.stack([deg[m].sum(0) for m in members])
    mem = [list(m) for m in members]
    for _ in range(iters):
        over = np.argwhere(wsum[:last] > CAP)
        if len(over) == 0:
            break
        w, t = over[0]
        cand = sorted(mem[w], key=lambda n: -deg[n][t])
        done = False
        for nd in cand[:20]:
            dn = deg[nd]
            for w2 in range(last):
                if w2 == w:
                    continue
                for nd2 in sorted(mem[w2], key=lambda n: deg[n][t])[:20]:
                    dn2 = deg[nd2]
                    ns_w = wsum[w] - dn + dn2
                    ns_w2 = wsum[w2] - dn2 + dn
                    if (ns_w <= CAP).all() and (ns_w2 <= CAP).all():
                        mem[w].remove(nd); mem[w].append(nd2)
                        mem[w2].remove(nd2); mem[w2].append(nd)
                        wsum[w] = ns_w; wsum[w2] = ns_w2
                        done = True
                        break
                if done:
                    break
            if done:
                break
        if not done:
            for nd in cand[:20]:
                dn = deg[nd]
                for nd2 in sorted(mem[last], key=lambda n: deg[n][t])[:40]:
                    dn2 = deg[nd2]
                    ns_w = wsum[w] - dn + dn2
                    if (ns_w <= CAP).all():
                        mem[w].remove(nd); mem[w].append(nd2)
                        mem[last].remove(nd2); mem[last].append(nd)
                        wsum[last] += dn - dn2
                        wsum[w] = ns_w
                        done = True
                        break
                if done:
                    break
        if not done:
            break
    return [np.array(m, np.int64) for m in mem]


def _prep(node_features, edge_index, edge_type, Wp, bp, Wm, bm, Wih, Whh, bih, bhh):
    """Host-side sharding/packing. Returns (meta, in_maps)."""
    x = np.asarray(node_features, np.float32)
    B = x.shape[0]
    N = B * MAXN
    GPC = B // NCORES             # graphs per core
    NB = GPC * MAXN               # nodes per core
    NWIN = GPC * WPG              # windows per core
    src = np.asarray(edge_index[0]).astype(np.int64)
    dst = np.asarray(edge_index[1]).astype(np.int64)
    et = np.asarray(edge_type).astype(np.int64)

    # per-(node, type) in-degree
    cnt = np.zeros((N, T), np.int64)
    np.add.at(cnt, (dst, et), 1)

    # balance windows within each graph -> node permutation
    old2new = np.empty(N, np.int64)
    for g in range(B):
        deg_g = cnt[g * MAXN:(g + 1) * MAXN]
        mem = _repair(_balance_graph(deg_g), deg_g)
        for w in range(WPG):
            pos = g * MAXN + w * WIN + np.arange(WIN)
            old2new[g * MAXN + mem[w]] = pos
    new2old = np.argsort(old2new)

    src_n = old2new[src]
    dst_n = old2new[dst]

    # group edges per core: key = ((gslot*WPG + w)*T + t)
    core = dst_n // NB
    rel = dst_n % NB
    win_in_core = rel // WIN      # 0..NWIN-1  (gslot*WPG + w)
    col = rel % WIN
    key = win_in_core * T + et
    NGRP = NWIN * T

    gsizes = np.zeros((NCORES, NGRP), np.int64)
    for c in range(NCORES):
        m = core == c
        gsizes[c] = np.bincount(key[m], minlength=NGRP)
    budget = np.ceil(gsizes.max(axis=0) / 128).astype(np.int64)  # chunks per group
    budget = np.maximum(budget, 1)
    ctot = int(budget.sum())
    ngg = (ctot + GSZ - 1) // GSZ      # gather groups of GSZ chunks
    ctotP = ngg * GSZ
    nslots = ctotP * 128
    gbase = np.concatenate([[0], np.cumsum(budget)])[:-1] * 128  # slot base per group

    # per-core slot arrays
    idx_maps, smat_maps = [], []
    counts_maps, xT_maps = [], []
    for c in range(NCORES):
        m = core == c
        kc, cc, sc = key[m], col[m], src_n[m]
        order = np.argsort(kc, kind="stable")
        kc, cc, sc = kc[order], cc[order], sc[order]
        # rank within group
        grp_start = np.searchsorted(kc, np.arange(NGRP), side="left")
        rank = np.arange(kc.size) - grp_start[kc]
        slot = gbase[kc] + rank
        src16 = np.zeros(nslots, np.int16)
        scol = np.full(nslots, -1, np.int64)
        src16[slot] = sc.astype(np.int16)
        scol[slot] = cc
        # idx: wrapped [16, nslots/16] replicated to 128 partitions
        idx = np.tile(src16.reshape(nslots // 16, 16).T, (8, 1)).copy()
        idx_maps.append(idx)
        # one-hot S: [ngg, 128, GSZ, 128] fp8 (0/1 exact)
        smat = np.zeros((ctotP * 128, WIN), F8)
        valid = scol >= 0
        smat[np.nonzero(valid)[0], scol[valid]] = 1
        smat = smat.reshape(ngg, GSZ, 128, WIN)
        smat = np.ascontiguousarray(smat.transpose(0, 2, 1, 3))  # [ngg,128,GSZ,128]
        smat_maps.append(smat)
        # counts (new order), [T, NB] bf16
        cslice = cnt[new2old[c * NB:(c + 1) * NB]]
        counts_maps.append(np.ascontiguousarray(cslice.T).astype(BF16))
        # xT [128, 2, NB] bf16: [p, k, node] = x[node, k*128+p]
        xs = x.reshape(N, F)[new2old[c * NB:(c + 1) * NB]]
        xp = np.zeros((NB, 2 * 128), np.float32)
        xp[:, :F] = xs
        xT = np.ascontiguousarray(xp.reshape(NB, 2, 128).transpose(2, 1, 0))
        xT_maps.append(xT.astype(BF16))

    # full permuted x as the layer-0 gather table (F padded to 256); by
    # linearity layer 0 aggregates raw x rows and the message matmul uses
    # Wm[0] @ Wp (weight folding), so no AllGather is needed for layer 0.
    # fp8: gathered-row quantization noise is averaged out by the 2048-wide
    # message contraction (~0.1% effect on m), so the h/x tables, S one-hots
    # and aggregation matmuls all run in fp8e4m3.
    xtbl = np.zeros((N, 2 * 128), np.float32)
    xtbl[:, :F] = x.reshape(N, F)[new2old]
    xtbl = xtbl.astype(F8)

    # weights (shared across cores)
    Wp = np.asarray(Wp, np.float32); bp_ = np.asarray(bp, np.float32)
    Wm_ = np.asarray(Wm, np.float32); bm_ = np.asarray(bm, np.float32)
    Wih_ = np.asarray(Wih, np.float32); Whh_ = np.asarray(Whh, np.float32)
    bih_ = np.asarray(bih, np.float32); bhh_ = np.asarray(bhh, np.float32)

    wpT = np.zeros((128, 2, H), np.float32)          # [p, fk, h']
    wpt = Wp.T                                       # [F, H]
    wpT[:, 0, :] = wpt[0:128]
    wpT[:F - 128, 1, :] = wpt[128:F]
    wp_in = wpT.astype(BF16)
    bp_in = np.ascontiguousarray(bp_.reshape(2, 128).T)          # [128, 2]

    # fold the input projection into layer 0's message weights: layer 0
    # aggregates raw x rows, so
    #   Wm0p[t,f,e] = sum_d Wm[0,t,e,d] Wp[d,f],  bm0p[t] = Wm[0,t] @ bp + bm[0,t]
    WmIN = np.zeros((L, T, 2 * 128, H), np.float32)   # [L, T, in(padded), out]
    WmIN[1:, :, :H, :] = Wm_[1:].transpose(0, 1, 3, 2)
    WmIN[0, :, :F, :] = np.einsum('ted,df->tfe', Wm_[0], Wp)
    bm_2 = bm_.copy()
    bm_2[0] = bm_[0] + np.einsum('ted,d->te', Wm_[0], bp_)
    bm_in = bm_2.astype(BF16)                         # [L, T, H]
    wm_in = np.ascontiguousarray(                     # [L, 128, 2, T, H]
        WmIN.reshape(L, T, 2, 128, H).transpose(0, 3, 2, 1, 4)).astype(BF16)
    wih_in = np.ascontiguousarray(                    # [L, 128, 2, 3H]
        Wih_.transpose(0, 2, 1).reshape(L, 2, 128, 3 * H).transpose(0, 2, 1, 3)
    ).astype(BF16)
    whh_in = np.ascontiguousarray(
        Whh_.transpose(0, 2, 1).reshape(L, 2, 128, 3 * H).transpose(0, 2, 1, 3)
    ).astype(BF16)
    brz = bih_[:, :2 * H] + bhh_[:, :2 * H]
    brz_in = np.ascontiguousarray(brz.reshape(L, 4, 128).transpose(0, 2, 1))  # [L,128,4]
    bin_in = np.ascontiguousarray(bih_[:, 2 * H:].reshape(L, 2, 128).transpose(0, 2, 1))
    bhn_in = np.ascontiguousarray(bhh_[:, 2 * H:].reshape(L, 2, 128).transpose(0, 2, 1))
    id128 = np.eye(128, dtype=BF16)

    in_maps = []
    for c in range(NCORES):
        in_maps.append({
            "xT": xT_maps[c], "idx": idx_maps[c], "smat": smat_maps[c],
            "countsT": counts_maps[c], "xtbl": xtbl,
            "wpT": wp_in, "bp": bp_in, "wmT": wm_in, "bmT": bm_in,
            "wihT": wih_in, "whhT": whh_in,
            "brz": brz_in, "bin_": bin_in, "bhn": bhn_in, "id128": id128,
        })
    meta = dict(B=B, N=N, GPC=GPC, NB=NB, NWIN=NWIN,
                budget=budget.reshape(NWIN, T), ctot=ctot, ngg=ngg,
                new2old=new2old)
    return meta, in_maps


def _build(meta, debug=False, skip=()):
    """Build the SPMD Bass program (identical across cores)."""
    skip = frozenset(skip)
    dt = mybir.dt
    N, NB, GPC, NWIN = meta["N"], meta["NB"], meta["GPC"], meta["NWIN"]
    budget, ngg = meta["budget"], meta["ngg"]
    ctotP = ngg * GSZ
    SLOT16 = ctotP * 128 // 16

    nc = bacc.Bacc("TRN2", target_bir_lowering=False, debug=False,
                   enable_asserts=False, num_devices=NCORES,
                   num_swdge_queues=4)

    # ---- I/O
    xT_in = nc.dram_tensor("xT", [128, 2, NB], dt.bfloat16, kind="ExternalInput").ap()
    xtbl_in = nc.dram_tensor("xtbl", [N, 2 * 128], dt.float8e4, kind="ExternalInput").ap()
    idx_in = nc.dram_tensor("idx", [128, SLOT16], dt.int16, kind="ExternalInput").ap()
    smat_in = nc.dram_tensor("smat", [ngg, 128, GSZ, WIN], dt.float8e4, kind="ExternalInput").ap()
    counts_in = nc.dram_tensor("countsT", [T, NB], dt.bfloat16, kind="ExternalInput").ap()
    wp_in = nc.dram_tensor("wpT", [128, 2, H], dt.bfloat16, kind="ExternalInput").ap()
    bp_in = nc.dram_tensor("bp", [128, 2], dt.float32, kind="ExternalInput").ap()
    wm_in = nc.dram_tensor("wmT", [L, 128, 2, T, H], dt.bfloat16, kind="ExternalInput").ap()
    bm_in = nc.dram_tensor("bmT", [L, T, H], dt.bfloat16, kind="ExternalInput").ap()
    wih_in = nc.dram_tensor("wihT", [L, 128, 2, 3 * H], dt.bfloat16, kind="ExternalInput").ap()
    whh_in = nc.dram_tensor("whhT", [L, 128, 2, 3 * H], dt.bfloat16, kind="ExternalInput").ap()
    brz_in = nc.dram_tensor("brz", [L, 128, 4], dt.float32, kind="ExternalInput").ap()
    bin_in = nc.dram_tensor("bin_", [L, 128, 2], dt.float32, kind="ExternalInput").ap()
    bhn_in = nc.dram_tensor("bhn", [L, 128, 2], dt.float32, kind="ExternalInput").ap()
    id_in = nc.dram_tensor("id128", [128, 128], dt.bfloat16, kind="ExternalInput").ap()
    out_t = nc.dram_tensor("outT", [2, 128, GPC], dt.float32, kind="ExternalOutput").ap()

    groups = [list(range(NCORES))]

    with tile.TileContext(nc) as tc:
        with (
            tc.tile_pool(name="per", bufs=1) as per,       # persistent SBUF
            tc.tile_pool(name="wts", bufs=2) as wts,       # per-layer weights
            tc.tile_pool(name="gth", bufs=3) as gth,       # gather/S stream
            tc.tile_pool(name="wrk", bufs=2) as wrk,       # A/mT/staging
            tc.tile_pool(name="gru", bufs=6) as grup,      # GRU temps
            tc.tile_pool(name="ps", bufs=1, space="PSUM") as ps,
            tc.tile_pool(name="dram", bufs=2, space="DRAM") as dram,
        ):
            # persistent loads
            idx_sb = per.tile([128, SLOT16], dt.int16)
            nc.sync.dma_start(idx_sb[:], idx_in[:])
            counts_sb = per.tile([T, NB], dt.bfloat16)
            nc.sync.dma_start(counts_sb[:], counts_in[:])
            wp_sb = per.tile([128, 2, H], dt.bfloat16)
            nc.sync.dma_start(wp_sb[:], wp_in[:])
            bp_sb = per.tile([128, 2], dt.float32)
            nc.sync.dma_start(bp_sb[:], bp_in[:])
            id_sb = per.tile([128, 128], dt.bfloat16)
            nc.sync.dma_start(id_sb[:], id_in[:])
            xT_sb = per.tile([128, 2, NB], dt.bfloat16)
            nc.sync.dma_start(xT_sb[:], xT_in[:])
            hT_sb = per.tile([128, 2, NB], dt.bfloat16)
            outsb = per.tile([128, 2, GPC], dt.float32)
            nc.vector.memset(outsb[:], 0.0)
            # one shared register for every gather's num_idxs (saves a per-call
            # MOVE on the gpsimd queue)
            nidx_reg = nc.gpsimd.to_reg(GSZ * 128)

            # agin/tbl DRAM tiles per stage (after-l0, after-l1); layer 0
            # gathers straight from the xtbl input, so no stage for it.
            agins = [dram.tile([NB, H], dt.float8e4, tag="agin", name=f"agin{i}")
                     for i in range(L - 1)]
            tbls = [dram.tile([N, H], dt.float8e4, tag="tbl", addr_space="Shared",
                              name=f"tbl{i}") for i in range(L - 1)]

            HWPG = WPG // 2                     # windows per half-graph

            def stage_half(q, half, stage_i):
                """Transpose a half-graph's h windows to node-major and DMA into
                agins[stage_i]; fire graph q's AllGather after its last half so
                graphs 0..GPC-2's exchanges overlap the layer's gather stream
                and only graph GPC-1's sits at the layer boundary."""
                stg = wrk.tile([128, HWPG, H], dt.float8e4, tag="stg", bufs=2)
                for wl in range(HWPG):
                    w = q * WPG + half * HWPG + wl
                    for hc in range(2):
                        tp = ps.tile([128, 128], dt.bfloat16, tag="tp", bufs=1)
                        nc.tensor.transpose(tp[:], hT_sb[:, hc, w * 128:(w + 1) * 128],
                                            id_sb[:])
                        nc.scalar.copy(stg[:, wl, hc * 128:(hc + 1) * 128], tp[:])
                dst_ap = agins[stage_i].rearrange("(w p) h -> p w h", p=128)
                wb = q * WPG + half * HWPG
                nc.sync.dma_start(dst_ap[:, wb:wb + HWPG, :], stg[:])
                if half == 1 and q == GPC - 1:
                    if "ag" not in skip:
                        nc.gpsimd.collective_compute(
                            "AllGather", mybir.AluOpType.bypass,
                            replica_groups=groups,
                            ins=[agins[stage_i].opt()], outs=[tbls[stage_i].opt()])
                    else:
                        nc.sync.dma_start(tbls[stage_i][0:NB], agins[stage_i][:])

            # ---- input projection: hT = Wp @ xT + bp (local h only; layer 0's
            # table is the xtbl input, so nothing to stage here)
            for s in range(NB // 512):
                for hm in range(2):
                    pm = ps.tile([128, 512], dt.float32, tag="mT", bufs=2)
                    nc.tensor.matmul(pm[:], wp_sb[:, 0, hm * 128:(hm + 1) * 128],
                                     xT_sb[:, 0, s * 512:(s + 1) * 512],
                                     start=True, stop=False)
                    nc.tensor.matmul(pm[:], wp_sb[:, 1, hm * 128:(hm + 1) * 128],
                                     xT_sb[:, 1, s * 512:(s + 1) * 512],
                                     start=False, stop=True)
                    nc.vector.tensor_scalar_add(hT_sb[:, hm, s * 512:(s + 1) * 512],
                                                pm[:], bp_sb[:, hm:hm + 1])

            for l in range(L):
                tbl = xtbl_in if l == 0 else tbls[l - 1]
                # ---- layer weights
                wm_sb = wts.tile([128, 2, T, H], dt.bfloat16, tag="wm")
                nc.sync.dma_start(wm_sb[:], wm_in[l])
                bm_sb = wts.tile([T, H], dt.bfloat16, tag="bm")
                nc.sync.dma_start(bm_sb[:], bm_in[l])
                wih_sb = wts.tile([128, 2, 3 * H], dt.bfloat16, tag="wih")
                nc.sync.dma_start(wih_sb[:], wih_in[l])
                whh_sb = wts.tile([128, 2, 3 * H], dt.bfloat16, tag="whh")
                nc.sync.dma_start(whh_sb[:], whh_in[l])
                brz_sb = wts.tile([128, 4], dt.float32, tag="brz")
                nc.sync.dma_start(brz_sb[:], brz_in[l])
                bin_sb = wts.tile([128, 2], dt.float32, tag="bin")
                nc.sync.dma_start(bin_sb[:], bin_in[l])
                bhn_sb = wts.tile([128, 2], dt.float32, tag="bhn")
                nc.sync.dma_start(bhn_sb[:], bhn_in[l])

                # ---- aggregation + message + GRU, one graph (1024 nodes) at a time
                cglob = 0          # global chunk counter (program order)
                gg_tiles = {}      # gather-group -> (G, S)

                def need(c, l=l, tbl=tbl, gg_tiles=gg_tiles):
                    gg = c // GSZ
                    while len(gg_tiles) == 0 or max(gg_tiles) < gg:
                        g_ = 0 if not gg_tiles else max(gg_tiles) + 1
                        Gt = gth.tile([128, GSZ, H], dt.float8e4, tag="G", bufs=8,
                                      name=f"G_{l}_{g_}")
                        if "gather" not in skip:
                            # round-robin the 4 SWDGE contexts: descriptor
                            # generation for up to 4 gathers proceeds in
                            # parallel (~4x Pool-engine throughput)
                            nc.gpsimd.dma_gather(
                                Gt[:], tbl[:],
                                idx_sb[:, g_ * GSZ * 8:(g_ + 1) * GSZ * 8],
                                num_idxs=GSZ * 128, num_idxs_reg=nidx_reg,
                                elem_size=H, queue_num=g_ % 4)
                        else:
                            nc.sync.dma_start(
                                Gt[:],
                                tbl[0:GSZ * 128].rearrange("(c p) h -> p c h", p=128))
                        St = gth.tile([128, GSZ, WIN], dt.float8e4, tag="S", bufs=8,
                                      name=f"S_{l}_{g_}")
                        if "sload" not in skip:
                            nc.sync.dma_start(St[:], smat_in[g_])
                        else:
                            nc.sync.dma_start(St[:], smat_in[0])
                        gg_tiles[g_] = (Gt, St)
                        if len(gg_tiles) > 6:
                            del gg_tiles[min(gg_tiles)]
                    return gg_tiles[gg], c % GSZ

                for q in range(GPC):
                    for half in range(2):
                        # per-half A with two buffers: the next half's PSUM
                        # copies need not wait for this half's message matmuls
                        # to finish reading (same total SBUF as one per-graph A)
                        A_sb = wrk.tile([128, T, 2, HWPG, WIN], dt.bfloat16,
                                        tag="A", bufs=2)
                        for wl in range(half * HWPG, (half + 1) * HWPG):
                            w = q * WPG + wl
                            for th in range(T // 2):
                                pa = ps.tile([128, 512], dt.float32, tag="agg", bufs=3)
                                for ti in range(2):
                                    t = th * 2 + ti
                                    nchunks = int(budget[w, t])
                                    for hc in range(2):
                                        off = (ti * 2 + hc) * 128
                                        ci = 0
                                        while ci < nchunks:
                                            (Gt, St), j = need(cglob + ci)
                                            if "aggmm" in skip:
                                                ci += 1
                                                continue
                                            # fp8 DoubleRow: two 128-deep
                                            # k-tiles per pass when the pair
                                            # sits in one gather-group tile
                                            if ci + 1 < nchunks and j + 1 < GSZ:
                                                need(cglob + ci + 1)
                                                nc.tensor.matmul(
                                                    pa[:, off:off + 128],
                                                    Gt[:, j:j + 2, hc * 128:(hc + 1) * 128],
                                                    St[:, j:j + 2, :],
                                                    start=(ci == 0),
                                                    stop=(ci + 2 >= nchunks),
                                                    perf_mode=mybir.MatmulPerfMode.DoubleRow)
                                                ci += 2
                                            else:
                                                nc.tensor.matmul(
                                                    pa[:, off:off + 128],
                                                    Gt[:, j, hc * 128:(hc + 1) * 128],
                                                    St[:, j, :],
                                                    start=(ci == 0),
                                                    stop=(ci == nchunks - 1))
                                                ci += 1
                                    cglob += nchunks
                                dst_ap = A_sb[:, th * 2:th * 2 + 2, :,
                                              wl - half * HWPG, :]
                                src_ap = pa.rearrange("p (t c k) -> p t c k", t=2, c=2)
                                if "aggcp" not in skip:
                                    if th % 2 == 0:
                                        nc.scalar.copy(dst_ap, src_ap)
                                    else:
                                        nc.vector.tensor_copy(dst_ap, src_ap)

                        # ---- message matmuls for this half: mT = sum_t WmT[t] @ A_t
                        mT_sb = wrk.tile([128, 2, 512], dt.bfloat16, tag="mT")
                        nbase = q * MAXN + half * 512
                        for hm in range(2):
                            pm = ps.tile([128, 512], dt.float32, tag="mT", bufs=2)
                            if "wt" not in skip:
                                nc.tensor.matmul(
                                    pm[:], bm_sb[:, hm * 128:(hm + 1) * 128],
                                    counts_sb[:, nbase:nbase + 512],
                                    start=True, stop=False)
                                for t in range(T):
                                    for hk in range(2):
                                        nc.tensor.matmul(
                                            pm[:],
                                            wm_sb[:, hk, t, hm * 128:(hm + 1) * 128],
                                            A_sb[:, t, hk, :, :],
                                            start=False, stop=(t == T - 1 and hk == 1))
                                nc.vector.tensor_copy(mT_sb[:, hm, :], pm[:])

                        # ---- GRU for this half's 512 nodes
                        if "gru" in skip:
                            continue
                        nsl = slice(nbase, nbase + 512)
                        r_sb = grup.tile([128, 2, 512], dt.float32, tag="r", bufs=2)
                        z_sb = grup.tile([128, 2, 512], dt.float32, tag="z", bufs=2)
                        for gm in range(4):
                            pg = ps.tile([128, 512], dt.float32, tag="gru", bufs=2)
                            gsl = slice(gm * 128, (gm + 1) * 128)
                            nc.tensor.matmul(pg[:], wih_sb[:, 0, gsl], mT_sb[:, 0, :],
                                             start=True, stop=False)
                            nc.tensor.matmul(pg[:], wih_sb[:, 1, gsl], mT_sb[:, 1, :],
                                             start=False, stop=False)
                            nc.tensor.matmul(pg[:], whh_sb[:, 0, gsl], hT_sb[:, 0, nsl],
                                             start=False, stop=False)
                            nc.tensor.matmul(pg[:], whh_sb[:, 1, gsl], hT_sb[:, 1, nsl],
                                             start=False, stop=True)
                            dst = r_sb[:, gm, :] if gm < 2 else z_sb[:, gm - 2, :]
                            nc.scalar.activation(dst, pg[:],
                                                 mybir.ActivationFunctionType.Sigmoid,
                                                 bias=brz_sb[:, gm:gm + 1])
                        nns, zds = [], []
                        for hc in range(2):
                            gsl = slice((4 + hc) * 128, (5 + hc) * 128)
                            ph = ps.tile([128, 512], dt.float32, tag="gru", bufs=2)
                            nc.tensor.matmul(ph[:], whh_sb[:, 0, gsl], hT_sb[:, 0, nsl],
                                             start=True, stop=False)
                            nc.tensor.matmul(ph[:], whh_sb[:, 1, gsl], hT_sb[:, 1, nsl],
                                             start=False, stop=True)
                            hnb = grup.tile([128, 512], dt.float32, tag="gt", bufs=4)
                            nc.vector.tensor_scalar_add(hnb[:], ph[:], bhn_sb[:, hc:hc + 1])
                            rhn = grup.tile([128, 512], dt.float32, tag="gt", bufs=4)
                            nc.vector.tensor_mul(rhn[:], r_sb[:, hc, :], hnb[:])
                            pi = ps.tile([128, 512], dt.float32, tag="gru", bufs=2)
                            nc.tensor.matmul(pi[:], wih_sb[:, 0, gsl], mT_sb[:, 0, :],
                                             start=True, stop=False)
                            nc.tensor.matmul(pi[:], wih_sb[:, 1, gsl], mT_sb[:, 1, :],
                                             start=False, stop=True)
                            tsum = grup.tile([128, 512], dt.float32, tag="gt", bufs=4)
                            nc.vector.tensor_add(tsum[:], pi[:], rhn[:])
                            nn = grup.tile([128, 512], dt.float32, tag="nnb", bufs=3)
                            nc.scalar.activation(nn[:], tsum[:],
                                                 mybir.ActivationFunctionType.Tanh,
                                                 bias=bin_sb[:, hc:hc + 1])
                            d_ = grup.tile([128, 512], dt.float32, tag="gt", bufs=4)
                            nc.vector.tensor_sub(d_[:], hT_sb[:, hc, nsl], nn[:])
                            zd = grup.tile([128, 512], dt.float32, tag="zdb", bufs=3)
                            nc.vector.tensor_mul(zd[:], z_sb[:, hc, :], d_[:])
                            nns.append(nn)
                            zds.append(zd)
                        # write h only after BOTH halves' matmuls consumed h_l
                        for hc in range(2):
                            if l < L - 1:
                                nc.vector.tensor_add(hT_sb[:, hc, nsl], nns[hc][:], zds[hc][:])
                            else:
                                hf = grup.tile([128, 512], dt.float32, tag="hf", bufs=2)
                                nc.vector.tensor_add(hf[:], nns[hc][:], zds[hc][:])
                                rs = grup.tile([128, 1], dt.float32, tag="rs", bufs=16)
                                nc.vector.tensor_reduce(rs[:], hf[:],
                                                        axis=mybir.AxisListType.X,
                                                        op=mybir.AluOpType.add)
                                if half == 0:
                                    nc.vector.tensor_copy(outsb[:, hc, q:q + 1], rs[:])
                                else:
                                    nc.vector.tensor_add(outsb[:, hc, q:q + 1],
                                                         outsb[:, hc, q:q + 1], rs[:])
                        # stage this half's new h for the next layer's table
                        if l < L - 1:
                            stage_half(q, half, l)
                assert cglob == int(budget.sum()), (cglob, int(budget.sum()))

            # ---- readout
            nc.sync.dma_start(out_t.rearrange("c p g -> p c g"), outsb[:])

    nc.compile()
    return nc


def kernel(**inputs):
    meta, in_maps = _prep(**inputs)
    nc = _build(meta)
    res = run_bass_kernel_spmd(nc, in_maps, core_ids=list(range(NCORES)))
    GPC = meta["GPC"]
    out = np.zeros((meta["B"], H), np.float32)
    for c in range(NCORES):
        ot = res.results[c]["outT"]          # [2, 128, GPC]
        for g in range(GPC):
            out[c * GPC + g] = np.concatenate([ot[0, :, g], ot[1, :, g]])
    return out



# revision 31
# speedup vs baseline: 2.2672x; 1.0511x over previous
"""BatchGGNNEncoder Trainium2 kernel: 8-core SPMD, dst-sharded message passing.

Full inputs in, full output out. Internally:
  - core c owns nodes [c*4096, (c+1)*4096) = graphs [4c, 4c+4) (data parallel).
  - aggregate-first GGNN layer:
        A_t[v] = sum_{e: dst=v, type=t} h[src_e]         (one-hot matmuls, PSUM)
        m      = sum_t A_t @ Wm[t].T + counts_t * bm[t]  (dense matmuls)
        h      = GRU(m, h)                               (matmuls + DVE/ACT)
  - h table (bf16, node-major) lives in DRAM, AllGathered across cores per layer;
    per-edge h[src] rows fetched with dma_gather (the kernel's critical path:
    ~8.4ns/edge of Q7 descriptor generation).
  - staging (transpose to node-major + DMA) for layer l+1's table is fused into
    layer l's per-graph GRU tail so the AllGather fires as early as possible.
  - nodes are permuted within each graph to balance (type, 128-dst-window) group
    sizes so the compiled program structure is identical on all 8 cores.
"""
import numpy as np
import ml_dtypes

import concourse.bass as bass
import concourse.bacc as bacc
import concourse.mybir as mybir
import concourse.tile as tile
from concourse.bass_utils import run_bass_kernel_spmd

BF16 = ml_dtypes.bfloat16
F8 = ml_dtypes.float8_e4m3

# problem constants (hardcoded per harness contract)
MAXN, F, H, T, L = 1024, 215, 256, 8, 3
NCORES = 8
WIN = 128                     # dst window (one-hot free width)
WPG = MAXN // WIN             # 8 windows per graph
GSZ = 8                       # chunks per dma_gather (8*128=1024 idxs; the SWDGE
                              # ring holds 64 m2s + 64 s2m pairs per engine, so
                              # 1024 idxs is the hard maximum per call)


def _balance_graph(deg):
    """Assign 1024 nodes (deg: [1024, T] type-degrees) to 8 windows of 128.
    Window WPG-1 takes the heaviest 128 nodes (the graph's excess, ~3 chunks
    per type); the remaining 896 are balanced across windows 0..WPG-2 under a
    hard 256 cap per type (2 chunks), with real slack since the heavy nodes
    are gone. Keeps cross-core max budgets at 2 for most groups."""
    tot = deg.sum(1)
    order = np.argsort(-tot, kind="stable")
    last = WPG - 1
    wsum = np.zeros((WPG, T), np.float64)
    wcnt = np.zeros(WPG, np.int64)
    members = [[] for _ in range(WPG)]
    CAP, CAP7 = 256.0, 381.0
    rest = []
    for nd in order:
        if wcnt[last] < 128 and ((wsum[last] + deg[nd]) <= CAP7).all():
            members[last].append(nd)
            wsum[last] += deg[nd]
            wcnt[last] += 1
        else:
            rest.append(nd)
    for nd in rest:
        d = deg[nd]
        ns = wsum[:last] + d
        feas = (wcnt[:last] < 128) & (ns <= CAP).all(axis=1)
        if feas.any():
            load = np.where(feas, ns.max(axis=1), np.inf)
            best = int(np.argmin(load))
        else:
            nsall = wsum + d
            dcost = (np.ceil(nsall / 128) - np.ceil(wsum / 128)).sum(axis=1)
            dcost[wcnt >= 128] = np.inf
            best = int(np.argmin(dcost))
        members[best].append(nd)
        wsum[best] += d
        wcnt[best] += 1
    return [np.array(m, np.int64) for m in members]


def _repair(members, deg, CAP=256.0, iters=4000):
    """Local-search swaps to push every (window<7, type) load under CAP so the
    cross-core budget max stays at 2 chunks outside the spill window."""
    last = WPG - 1
    deg = deg.astype(np.float64)
    wsum = np.stack([deg[m].sum(0) for m in members])
    mem = [list(m) for m in members]
    for _ in range(iters):
        over = np.argwhere(wsum[:last] > CAP)
        if len(over) == 0:
            break
        w, t = over[0]
        cand = sorted(mem[w], key=lambda n: -deg[n][t])
        done = False
        for nd in cand[:20]:
            dn = deg[nd]
            for w2 in range(last):
                if w2 == w:
                    continue
                for nd2 in sorted(mem[w2], key=lambda n: deg[n][t])[:20]:
                    dn2 = deg[nd2]
                    ns_w = wsum[w] - dn + dn2
                    ns_w2 = wsum[w2] - dn2 + dn
                    if (ns_w <= CAP).all() and (ns_w2 <= CAP).all():
                        mem[w].remove(nd); mem[w].append(nd2)
                        mem[w2].remove(nd2); mem[w2].append(nd)
                        wsum[w] = ns_w; wsum[w2] = ns_w2
                        done = True
                        break
                if done:
                    break
            if done:
                break
        if not done:
            for nd in cand[:20]:
                dn = deg[nd]
                for nd2 in sorted(mem[last], key=lambda n: deg[n][t])[:40]:
                    dn2 = deg[nd2]
                    ns_w = wsum[w] - dn + dn2
                    if (ns_w <= CAP).all():
                        mem[w].remove(nd); mem[w].append(nd2)
                        mem[last].remove(nd2); mem[last].append(nd)
                        wsum[last] += dn - dn2
                        wsum[w] = ns_w
                        done = True
                        break
                if done:
                    break
        if not done:
            break
    return [np.array(m, np.int64) for m in mem]


def _prep(node_features, edge_index, edge_type, Wp, bp, Wm, bm, Wih, Whh, bih, bhh):
    """Host-side sharding/packing. Returns (meta, in_maps)."""
    x = np.asarray(node_features, np.float32)
    B = x.shape[0]
    N = B * MAXN
    GPC = B // NCORES             # graphs per core
    NB = GPC * MAXN               # nodes per core
    NWIN = GPC * WPG              # windows per core
    src = np.asarray(edge_index[0]).astype(np.int64)
    dst = np.asarray(edge_index[1]).astype(np.int64)
    et = np.asarray(edge_type).astype(np.int64)

    # per-(node, type) in-degree
    cnt = np.zeros((N, T), np.int64)
    np.add.at(cnt, (dst, et), 1)

    # balance windows within each graph -> node permutation
    old2new = np.empty(N, np.int64)
    for g in range(B):
        deg_g = cnt[g * MAXN:(g + 1) * MAXN]
        mem = _repair(_balance_graph(deg_g), deg_g)
        for w in range(WPG):
            pos = g * MAXN + w * WIN + np.arange(WIN)
            old2new[g * MAXN + mem[w]] = pos
    new2old = np.argsort(old2new)

    src_n = old2new[src]
    dst_n = old2new[dst]

    # group edges per core: key = ((gslot*WPG + w)*T + t)
    core = dst_n // NB
    rel = dst_n % NB
    win_in_core = rel // WIN      # 0..NWIN-1  (gslot*WPG + w)
    col = rel % WIN
    key = win_in_core * T + et
    NGRP = NWIN * T

    gsizes = np.zeros((NCORES, NGRP), np.int64)
    for c in range(NCORES):
        m = core == c
        gsizes[c] = np.bincount(key[m], minlength=NGRP)
    budget = np.ceil(gsizes.max(axis=0) / 128).astype(np.int64)  # chunks per group
    budget = np.maximum(budget, 1)
    ctot = int(budget.sum())
    ngg = (ctot + GSZ - 1) // GSZ      # gather groups of GSZ chunks
    ctotP = ngg * GSZ
    nslots = ctotP * 128
    gbase = np.concatenate([[0], np.cumsum(budget)])[:-1] * 128  # slot base per group

    # per-core slot arrays
    idx_maps, smat_maps = [], []
    counts_maps, xT_maps = [], []
    for c in range(NCORES):
        m = core == c
        kc, cc, sc = key[m], col[m], src_n[m]
        order = np.argsort(kc, kind="stable")
        kc, cc, sc = kc[order], cc[order], sc[order]
        # rank within group
        grp_start = np.searchsorted(kc, np.arange(NGRP), side="left")
        rank = np.arange(kc.size) - grp_start[kc]
        slot = gbase[kc] + rank
        src16 = np.zeros(nslots, np.int16)
        scol = np.full(nslots, -1, np.int64)
        src16[slot] = sc.astype(np.int16)
        scol[slot] = cc
        # idx: wrapped [16, nslots/16] replicated to 128 partitions
        idx = np.tile(src16.reshape(nslots // 16, 16).T, (8, 1)).copy()
        idx_maps.append(idx)
        # one-hot S: [ngg, 128, GSZ, 128] fp8 (0/1 exact)
        smat = np.zeros((ctotP * 128, WIN), F8)
        valid = scol >= 0
        smat[np.nonzero(valid)[0], scol[valid]] = 1
        smat = smat.reshape(ngg, GSZ, 128, WIN)
        smat = np.ascontiguousarray(smat.transpose(0, 2, 1, 3))  # [ngg,128,GSZ,128]
        smat_maps.append(smat)
        # counts (new order), [T, NB] bf16
        cslice = cnt[new2old[c * NB:(c + 1) * NB]]
        counts_maps.append(np.ascontiguousarray(cslice.T).astype(BF16))
        # xT [128, 2, NB] bf16: [p, k, node] = x[node, k*128+p]
        xs = x.reshape(N, F)[new2old[c * NB:(c + 1) * NB]]
        xp = np.zeros((NB, 2 * 128), np.float32)
        xp[:, :F] = xs
        xT = np.ascontiguousarray(xp.reshape(NB, 2, 128).transpose(2, 1, 0))
        xT_maps.append(xT.astype(BF16))

    # full permuted x as the layer-0 gather table (F padded to 256); by
    # linearity layer 0 aggregates raw x rows and the message matmul uses
    # Wm[0] @ Wp (weight folding), so no AllGather is needed for layer 0.
    # fp8: gathered-row quantization noise is averaged out by the 2048-wide
    # message contraction (~0.1% effect on m), so the h/x tables, S one-hots
    # and aggregation matmuls all run in fp8e4m3.
    xtbl = np.zeros((N, 2 * 128), np.float32)
    xtbl[:, :F] = x.reshape(N, F)[new2old]
    xtbl = xtbl.astype(F8)

    # weights (shared across cores)
    Wp = np.asarray(Wp, np.float32); bp_ = np.asarray(bp, np.float32)
    Wm_ = np.asarray(Wm, np.float32); bm_ = np.asarray(bm, np.float32)
    Wih_ = np.asarray(Wih, np.float32); Whh_ = np.asarray(Whh, np.float32)
    bih_ = np.asarray(bih, np.float32); bhh_ = np.asarray(bhh, np.float32)

    wpT = np.zeros((128, 2, H), np.float32)          # [p, fk, h']
    wpt = Wp.T                                       # [F, H]
    wpT[:, 0, :] = wpt[0:128]
    wpT[:F - 128, 1, :] = wpt[128:F]
    wp_in = wpT.astype(BF16)
    bp_in = np.ascontiguousarray(bp_.reshape(2, 128).T)          # [128, 2]

    # fold the input projection into layer 0's message weights: layer 0
    # aggregates raw x rows, so
    #   Wm0p[t,f,e] = sum_d Wm[0,t,e,d] Wp[d,f],  bm0p[t] = Wm[0,t] @ bp + bm[0,t]
    WmIN = np.zeros((L, T, 2 * 128, H), np.float32)   # [L, T, in(padded), out]
    WmIN[1:, :, :H, :] = Wm_[1:].transpose(0, 1, 3, 2)
    WmIN[0, :, :F, :] = np.einsum('ted,df->tfe', Wm_[0], Wp)
    bm_2 = bm_.copy()
    bm_2[0] = bm_[0] + np.einsum('ted,d->te', Wm_[0], bp_)
    bm_in = bm_2.astype(BF16)                         # [L, T, H]
    wm_in = np.ascontiguousarray(                     # [L, 128, 2, T, H]
        WmIN.reshape(L, T, 2, 128, H).transpose(0, 3, 2, 1, 4)).astype(BF16)
    wih_in = np.ascontiguousarray(                    # [L, 128, 2, 3H]
        Wih_.transpose(0, 2, 1).reshape(L, 2, 128, 3 * H).transpose(0, 2, 1, 3)
    ).astype(BF16)
    whh_in = np.ascontiguousarray(
        Whh_.transpose(0, 2, 1).reshape(L, 2, 128, 3 * H).transpose(0, 2, 1, 3)
    ).astype(BF16)
    brz = bih_[:, :2 * H] + bhh_[:, :2 * H]
    brz_in = np.ascontiguousarray(brz.reshape(L, 4, 128).transpose(0, 2, 1))  # [L,128,4]
    bin_in = np.ascontiguousarray(bih_[:, 2 * H:].reshape(L, 2, 128).transpose(0, 2, 1))
    bhn_in = np.ascontiguousarray(bhh_[:, 2 * H:].reshape(L, 2, 128).transpose(0, 2, 1))
    id128 = np.eye(128, dtype=BF16)

    in_maps = []
    for c in range(NCORES):
        in_maps.append({
            "xT": xT_maps[c], "idx": idx_maps[c], "smat": smat_maps[c],
            "countsT": counts_maps[c], "xtbl": xtbl,
            "wpT": wp_in, "bp": bp_in, "wmT": wm_in, "bmT": bm_in,
            "wihT": wih_in, "whhT": whh_in,
            "brz": brz_in, "bin_": bin_in, "bhn": bhn_in, "id128": id128,
        })
    meta = dict(B=B, N=N, GPC=GPC, NB=NB, NWIN=NWIN,
                budget=budget.reshape(NWIN, T), ctot=ctot, ngg=ngg,
                new2old=new2old)
    return meta, in_maps


def _build(meta, debug=False, skip=()):
    """Build the SPMD Bass program (identical across cores)."""
    skip = frozenset(skip)
    dt = mybir.dt
    N, NB, GPC, NWIN = meta["N"], meta["NB"], meta["GPC"], meta["NWIN"]
    budget, ngg = meta["budget"], meta["ngg"]
    ctotP = ngg * GSZ
    SLOT16 = ctotP * 128 // 16

    nc = bacc.Bacc("TRN2", target_bir_lowering=False, debug=False,
                   enable_asserts=False, num_devices=NCORES,
                   num_swdge_queues=4)

    # ---- I/O
    xT_in = nc.dram_tensor("xT", [128, 2, NB], dt.bfloat16, kind="ExternalInput").ap()
    xtbl_in = nc.dram_tensor("xtbl", [N, 2 * 128], dt.float8e4, kind="ExternalInput").ap()
    idx_in = nc.dram_tensor("idx", [128, SLOT16], dt.int16, kind="ExternalInput").ap()
    smat_in = nc.dram_tensor("smat", [ngg, 128, GSZ, WIN], dt.float8e4, kind="ExternalInput").ap()
    counts_in = nc.dram_tensor("countsT", [T, NB], dt.bfloat16, kind="ExternalInput").ap()
    wp_in = nc.dram_tensor("wpT", [128, 2, H], dt.bfloat16, kind="ExternalInput").ap()
    bp_in = nc.dram_tensor("bp", [128, 2], dt.float32, kind="ExternalInput").ap()
    wm_in = nc.dram_tensor("wmT", [L, 128, 2, T, H], dt.bfloat16, kind="ExternalInput").ap()
    bm_in = nc.dram_tensor("bmT", [L, T, H], dt.bfloat16, kind="ExternalInput").ap()
    wih_in = nc.dram_tensor("wihT", [L, 128, 2, 3 * H], dt.bfloat16, kind="ExternalInput").ap()
    whh_in = nc.dram_tensor("whhT", [L, 128, 2, 3 * H], dt.bfloat16, kind="ExternalInput").ap()
    brz_in = nc.dram_tensor("brz", [L, 128, 4], dt.float32, kind="ExternalInput").ap()
    bin_in = nc.dram_tensor("bin_", [L, 128, 2], dt.float32, kind="ExternalInput").ap()
    bhn_in = nc.dram_tensor("bhn", [L, 128, 2], dt.float32, kind="ExternalInput").ap()
    id_in = nc.dram_tensor("id128", [128, 128], dt.bfloat16, kind="ExternalInput").ap()
    out_t = nc.dram_tensor("outT", [2, 128, GPC], dt.float32, kind="ExternalOutput").ap()

    groups = [list(range(NCORES))]

    with tile.TileContext(nc) as tc:
        with (
            tc.tile_pool(name="per", bufs=1) as per,       # persistent SBUF
            tc.tile_pool(name="wts", bufs=2) as wts,       # per-layer weights
            tc.tile_pool(name="gth", bufs=3) as gth,       # gather/S stream
            tc.tile_pool(name="wrk", bufs=2) as wrk,       # A/mT/staging
            tc.tile_pool(name="gru", bufs=6) as grup,      # GRU temps
            tc.tile_pool(name="ps", bufs=1, space="PSUM") as ps,
            tc.tile_pool(name="dram", bufs=2, space="DRAM") as dram,
        ):
            # persistent loads
            idx_sb = per.tile([128, SLOT16], dt.int16)
            nc.sync.dma_start(idx_sb[:], idx_in[:])
            counts_sb = per.tile([T, NB], dt.bfloat16)
            nc.sync.dma_start(counts_sb[:], counts_in[:])
            wp_sb = per.tile([128, 2, H], dt.bfloat16)
            nc.sync.dma_start(wp_sb[:], wp_in[:])
            bp_sb = per.tile([128, 2], dt.float32)
            nc.sync.dma_start(bp_sb[:], bp_in[:])
            id_sb = per.tile([128, 128], dt.bfloat16)
            nc.sync.dma_start(id_sb[:], id_in[:])
            xT_sb = per.tile([128, 2, NB], dt.bfloat16)
            nc.sync.dma_start(xT_sb[:], xT_in[:])
            hT_sb = per.tile([128, 2, NB], dt.bfloat16)
            outsb = per.tile([128, 2, GPC], dt.float32)
            nc.vector.memset(outsb[:], 0.0)
            # one shared register for every gather's num_idxs (saves a per-call
            # MOVE on the gpsimd queue)
            nidx_reg = nc.gpsimd.to_reg(GSZ * 128)

            # agin/tbl DRAM tiles per stage (after-l0, after-l1); layer 0
            # gathers straight from the xtbl input, so no stage for it.
            agins = [dram.tile([NB, H], dt.float8e4, tag="agin", name=f"agin{i}")
                     for i in range(L - 1)]
            tbls = [dram.tile([N, H], dt.float8e4, tag="tbl", addr_space="Shared",
                              name=f"tbl{i}") for i in range(L - 1)]

            HWPG = WPG // 2                     # windows per half-graph

            def stage_half(q, half, stage_i):
                """Transpose a half-graph's h windows to node-major and DMA into
                agins[stage_i]; fire graph q's AllGather after its last half so
                graphs 0..GPC-2's exchanges overlap the layer's gather stream
                and only graph GPC-1's sits at the layer boundary."""
                stg = wrk.tile([128, HWPG, H], dt.float8e4, tag="stg", bufs=2)
                for wl in range(HWPG):
                    w = q * WPG + half * HWPG + wl
                    for hc in range(2):
                        tp = ps.tile([128, 128], dt.bfloat16, tag="tp", bufs=1)
                        nc.tensor.transpose(tp[:], hT_sb[:, hc, w * 128:(w + 1) * 128],
                                            id_sb[:])
                        nc.scalar.copy(stg[:, wl, hc * 128:(hc + 1) * 128], tp[:])
                dst_ap = agins[stage_i].rearrange("(w p) h -> p w h", p=128)
                wb = q * WPG + half * HWPG
                nc.sync.dma_start(dst_ap[:, wb:wb + HWPG, :], stg[:])
                if half == 1 and q == GPC - 1:
                    if "ag" not in skip:
                        nc.gpsimd.collective_compute(
                            "AllGather", mybir.AluOpType.bypass,
                            replica_groups=groups,
                            ins=[agins[stage_i].opt()], outs=[tbls[stage_i].opt()])
                    else:
                        nc.sync.dma_start(tbls[stage_i][0:NB], agins[stage_i][:])

            # ---- input projection: hT = Wp @ xT + bp (local h only; layer 0's
            # table is the xtbl input, so nothing to stage here)
            for s in range(NB // 512):
                for hm in range(2):
                    pm = ps.tile([128, 512], dt.float32, tag="mT", bufs=2)
                    nc.tensor.matmul(pm[:], wp_sb[:, 0, hm * 128:(hm + 1) * 128],
                                     xT_sb[:, 0, s * 512:(s + 1) * 512],
                                     start=True, stop=False)
                    nc.tensor.matmul(pm[:], wp_sb[:, 1, hm * 128:(hm + 1) * 128],
                                     xT_sb[:, 1, s * 512:(s + 1) * 512],
                                     start=False, stop=True)
                    nc.vector.tensor_scalar_add(hT_sb[:, hm, s * 512:(s + 1) * 512],
                                                pm[:], bp_sb[:, hm:hm + 1])

            for l in range(L):
                tbl = xtbl_in if l == 0 else tbls[l - 1]
                # ---- layer weights
                wm_sb = wts.tile([128, 2, T, H], dt.bfloat16, tag="wm")
                nc.sync.dma_start(wm_sb[:], wm_in[l])
                bm_sb = wts.tile([T, H], dt.bfloat16, tag="bm")
                nc.sync.dma_start(bm_sb[:], bm_in[l])
                wih_sb = wts.tile([128, 2, 3 * H], dt.bfloat16, tag="wih")
                nc.sync.dma_start(wih_sb[:], wih_in[l])
                whh_sb = wts.tile([128, 2, 3 * H], dt.bfloat16, tag="whh")
                nc.sync.dma_start(whh_sb[:], whh_in[l])
                brz_sb = wts.tile([128, 4], dt.float32, tag="brz")
                nc.sync.dma_start(brz_sb[:], brz_in[l])
                bin_sb = wts.tile([128, 2], dt.float32, tag="bin")
                nc.sync.dma_start(bin_sb[:], bin_in[l])
                bhn_sb = wts.tile([128, 2], dt.float32, tag="bhn")
                nc.sync.dma_start(bhn_sb[:], bhn_in[l])

                # ---- aggregation + message + GRU, one graph (1024 nodes) at a time
                cglob = 0          # global chunk counter (program order)
                gg_tiles = {}      # gather-group -> (G, S)

                def need(c, l=l, tbl=tbl, gg_tiles=gg_tiles):
                    gg = c // GSZ
                    while len(gg_tiles) == 0 or max(gg_tiles) < gg:
                        g_ = 0 if not gg_tiles else max(gg_tiles) + 1
                        Gt = gth.tile([128, GSZ, H], dt.float8e4, tag="G", bufs=10,
                                      name=f"G_{l}_{g_}")
                        if "gather" not in skip:
                            # round-robin the 4 SWDGE contexts: descriptor
                            # generation for up to 4 gathers proceeds in
                            # parallel (~4x Pool-engine throughput)
                            nc.gpsimd.dma_gather(
                                Gt[:], tbl[:],
                                idx_sb[:, g_ * GSZ * 8:(g_ + 1) * GSZ * 8],
                                num_idxs=GSZ * 128, num_idxs_reg=nidx_reg,
                                elem_size=H, queue_num=g_ % 4)
                        else:
                            nc.sync.dma_start(
                                Gt[:],
                                tbl[0:GSZ * 128].rearrange("(c p) h -> p c h", p=128))
                        St = gth.tile([128, GSZ, WIN], dt.float8e4, tag="S", bufs=10,
                                      name=f"S_{l}_{g_}")
                        if "sload" not in skip:
                            nc.sync.dma_start(St[:], smat_in[g_])
                        else:
                            nc.sync.dma_start(St[:], smat_in[0])
                        gg_tiles[g_] = (Gt, St)
                        if len(gg_tiles) > 8:
                            del gg_tiles[min(gg_tiles)]
                    return gg_tiles[gg], c % GSZ

                for q in range(GPC):
                    for half in range(2):
                        # per-half A with two buffers: the next half's PSUM
                        # copies need not wait for this half's message matmuls
                        # to finish reading (same total SBUF as one per-graph A)
                        A_sb = wrk.tile([128, T, 2, HWPG, WIN], dt.bfloat16,
                                        tag="A", bufs=2)
                        for wl in range(half * HWPG, (half + 1) * HWPG):
                            w = q * WPG + wl
                            for th in range(T // 2):
                                pa = ps.tile([128, 512], dt.float32, tag="agg", bufs=3)
                                for ti in range(2):
                                    t = th * 2 + ti
                                    nchunks = int(budget[w, t])
                                    for hc in range(2):
                                        off = (ti * 2 + hc) * 128
                                        ci = 0
                                        while ci < nchunks:
                                            (Gt, St), j = need(cglob + ci)
                                            if "aggmm" in skip:
                                                ci += 1
                                                continue
                                            # fp8 DoubleRow: two 128-deep
                                            # k-tiles per pass when the pair
                                            # sits in one gather-group tile
                                            if ci + 1 < nchunks and j + 1 < GSZ:
                                                need(cglob + ci + 1)
                                                nc.tensor.matmul(
                                                    pa[:, off:off + 128],
                                                    Gt[:, j:j + 2, hc * 128:(hc + 1) * 128],
                                                    St[:, j:j + 2, :],
                                                    start=(ci == 0),
                                                    stop=(ci + 2 >= nchunks),
                                                    perf_mode=mybir.MatmulPerfMode.DoubleRow)
                                                ci += 2
                                            else:
                                                nc.tensor.matmul(
                                                    pa[:, off:off + 128],
                                                    Gt[:, j, hc * 128:(hc + 1) * 128],
                                                    St[:, j, :],
                                                    start=(ci == 0),
                                                    stop=(ci == nchunks - 1))
                                                ci += 1
                                    cglob += nchunks
                                dst_ap = A_sb[:, th * 2:th * 2 + 2, :,
                                              wl - half * HWPG, :]
                                src_ap = pa.rearrange("p (t c k) -> p t c k", t=2, c=2)
                                if "aggcp" not in skip:
                                    # all A copies on ACT: DVE shares its SBUF
                                    # port with the Q7 gather contexts, so
                                    # keeping DVE quiet speeds descriptor gen
                                    nc.scalar.copy(dst_ap, src_ap)

                        # ---- message matmuls for this half: mT = sum_t WmT[t] @ A_t
                        mT_sb = wrk.tile([128, 2, 512], dt.bfloat16, tag="mT")
                        nbase = q * MAXN + half * 512
                        for hm in range(2):
                            pm = ps.tile([128, 512], dt.float32, tag="mT", bufs=2)
                            if "wt" not in skip:
                                nc.tensor.matmul(
                                    pm[:], bm_sb[:, hm * 128:(hm + 1) * 128],
                                    counts_sb[:, nbase:nbase + 512],
                                    start=True, stop=False)
                                for t in range(T):
                                    for hk in range(2):
                                        nc.tensor.matmul(
                                            pm[:],
                                            wm_sb[:, hk, t, hm * 128:(hm + 1) * 128],
                                            A_sb[:, t, hk, :, :],
                                            start=False, stop=(t == T - 1 and hk == 1))
                                nc.vector.tensor_copy(mT_sb[:, hm, :], pm[:])

                        # ---- GRU for this half's 512 nodes
                        if "gru" in skip:
                            continue
                        nsl = slice(nbase, nbase + 512)
                        r_sb = grup.tile([128, 2, 512], dt.float32, tag="r", bufs=2)
                        z_sb = grup.tile([128, 2, 512], dt.float32, tag="z", bufs=2)
                        for gm in range(4):
                            pg = ps.tile([128, 512], dt.float32, tag="gru", bufs=2)
                            gsl = slice(gm * 128, (gm + 1) * 128)
                            nc.tensor.matmul(pg[:], wih_sb[:, 0, gsl], mT_sb[:, 0, :],
                                             start=True, stop=False)
                            nc.tensor.matmul(pg[:], wih_sb[:, 1, gsl], mT_sb[:, 1, :],
                                             start=False, stop=False)
                            nc.tensor.matmul(pg[:], whh_sb[:, 0, gsl], hT_sb[:, 0, nsl],
                                             start=False, stop=False)
                            nc.tensor.matmul(pg[:], whh_sb[:, 1, gsl], hT_sb[:, 1, nsl],
                                             start=False, stop=True)
                            dst = r_sb[:, gm, :] if gm < 2 else z_sb[:, gm - 2, :]
                            nc.scalar.activation(dst, pg[:],
                                                 mybir.ActivationFunctionType.Sigmoid,
                                                 bias=brz_sb[:, gm:gm + 1])
                        nns, zds = [], []
                        for hc in range(2):
                            gsl = slice((4 + hc) * 128, (5 + hc) * 128)
                            ph = ps.tile([128, 512], dt.float32, tag="gru", bufs=2)
                            nc.tensor.matmul(ph[:], whh_sb[:, 0, gsl], hT_sb[:, 0, nsl],
                                             start=True, stop=False)
                            nc.tensor.matmul(ph[:], whh_sb[:, 1, gsl], hT_sb[:, 1, nsl],
                                             start=False, stop=True)
                            hnb = grup.tile([128, 512], dt.float32, tag="gt", bufs=4)
                            nc.vector.tensor_scalar_add(hnb[:], ph[:], bhn_sb[:, hc:hc + 1])
                            rhn = grup.tile([128, 512], dt.float32, tag="gt", bufs=4)
                            nc.vector.tensor_mul(rhn[:], r_sb[:, hc, :], hnb[:])
                            pi = ps.tile([128, 512], dt.float32, tag="gru", bufs=2)
                            nc.tensor.matmul(pi[:], wih_sb[:, 0, gsl], mT_sb[:, 0, :],
                                             start=True, stop=False)
                            nc.tensor.matmul(pi[:], wih_sb[:, 1, gsl], mT_sb[:, 1, :],
                                             start=False, stop=True)
                            tsum = grup.tile([128, 512], dt.float32, tag="gt", bufs=4)
                            nc.vector.tensor_add(tsum[:], pi[:], rhn[:])
                            nn = grup.tile([128, 512], dt.float32, tag="nnb", bufs=3)
                            nc.scalar.activation(nn[:], tsum[:],
                                                 mybir.ActivationFunctionType.Tanh,
                                                 bias=bin_sb[:, hc:hc + 1])
                            d_ = grup.tile([128, 512], dt.float32, tag="gt", bufs=4)
                            nc.vector.tensor_sub(d_[:], hT_sb[:, hc, nsl], nn[:])
                            zd = grup.tile([128, 512], dt.float32, tag="zdb", bufs=3)
                            nc.vector.tensor_mul(zd[:], z_sb[:, hc, :], d_[:])
                            nns.append(nn)
                            zds.append(zd)
                        # write h only after BOTH halves' matmuls consumed h_l
                        for hc in range(2):
                            if l < L - 1:
                                nc.vector.tensor_add(hT_sb[:, hc, nsl], nns[hc][:], zds[hc][:])
                            else:
                                hf = grup.tile([128, 512], dt.float32, tag="hf", bufs=2)
                                nc.vector.tensor_add(hf[:], nns[hc][:], zds[hc][:])
                                rs = grup.tile([128, 1], dt.float32, tag="rs", bufs=16)
                                nc.vector.tensor_reduce(rs[:], hf[:],
                                                        axis=mybir.AxisListType.X,
                                                        op=mybir.AluOpType.add)
                                if half == 0:
                                    nc.vector.tensor_copy(outsb[:, hc, q:q + 1], rs[:])
                                else:
                                    nc.vector.tensor_add(outsb[:, hc, q:q + 1],
                                                         outsb[:, hc, q:q + 1], rs[:])
                        # stage this half's new h for the next layer's table
                        if l < L - 1:
                            stage_half(q, half, l)
                assert cglob == int(budget.sum()), (cglob, int(budget.sum()))

            # ---- readout
            nc.sync.dma_start(out_t.rearrange("c p g -> p c g"), outsb[:])

    nc.compile()
    return nc


def kernel(**inputs):
    meta, in_maps = _prep(**inputs)
    nc = _build(meta)
    res = run_bass_kernel_spmd(nc, in_maps, core_ids=list(range(NCORES)))
    GPC = meta["GPC"]
    out = np.zeros((meta["B"], H), np.float32)
    for c in range(NCORES):
        ot = res.results[c]["outT"]          # [2, 128, GPC]
        for g in range(GPC):
            out[c * GPC + g] = np.concatenate([ot[0, :, g], ot[1, :, g]])
    return out

